# revision 1
# baseline (speedup 1.0000x reference)
"""Trainium2 Bass kernel for nn_Encoder_66872640799015 (segment_reduce).

Recurrent conv encoder over 32768 pedestrians (4096 scenes x 8), 12 steps.
Sharding: data-parallel over scenes — 8 cores x 4096 pedestrians (512 whole
scenes per core), weights replicated.

Key algorithmic idea: each scan step shifts the conv input window by one
column, so all conv outputs except the newest position roll over from the
previous step.  Per step only ONE new conv position per layer is computed
(~4.8x FLOP reduction).  Rolling ring buffers live in SBUF for the whole
kernel; weight matrices are pre-permuted on the host for each of the 3 ring
rotations so no data movement is needed for the rolling.

Layout: channel-major (channels on partitions, pedestrians on the free dim).
Matmuls run as float32r (full-rate fp32 storage) for the conv stack and
bfloat16 for the small dec / rel heads.  The per-scene segment max is a
strided VectorE reduce over groups of 8 along the free dim.  The final
rels output for all 12 steps is computed at the end as a single M=24
block-matmul over the stored conv3 features.
"""

import sys

sys.path.insert(0, "/opt/trn_rl_repo")

import numpy as np
import ml_dtypes

import concourse.bass as bass
import concourse.bacc as bacc
import concourse.tile as tile
from concourse import mybir
from concourse.bass_utils import run_bass_kernel_spmd

NCORES = 8
BATCH = 32768
B = BATCH // NCORES        # pedestrians per core
T = 8                      # obs_len
SEQ = 12                   # seq_len
SCENE = 8                  # pedestrians per scene
NS = B // SCENE            # scenes per core
CH = 512                   # free-dim chunk (one PSUM bank of fp32)
NCHUNK = B // CH
NSLOT = SEQ // 2           # S_all free slots (2 steps per slot)

F32 = mybir.dt.float32
F32R = mybir.dt.float32r
BF16 = mybir.dt.bfloat16

_cache = {}

# engine-assignment knobs (chunks 0..NCHUNK-1): chunk < knob -> ScalarE(ACT),
# else VectorE(DVE)
R3_ACT = 8     # relu3-lo split
R2_ACT = 8     # relu2 split
R1_ACT = 8     # relu1 split
S1_ACT = 0     # stage-1 copy split
SEG_POOL = False   # segment-max on GpSimd instead of VectorE
HI_POOL = False    # conv3 hi-band dup copy on GpSimd
PSUM_BUFS = (3, 2, 1, 1)   # (pdec, pc1, pc2, pc3); wide bufs cost 2 banks
WIDE1 = False   # conv1 psum spans 2 banks -> one relu per chunk-pair
WIDE2 = True    # conv2 psum spans 2 banks -> one relu per chunk-pair


def _perm(r):
    """S-feature row (32*t + ch) -> reference feature index (2*ch + t)."""
    t, ch = r // 32, r % 32
    return 2 * ch + t


def _host_weights(W_se, b_se, v1, g1, b1, v2, g2, b2, v3, g3, b3, W_hp, b_hp):
    """Derive all device weight tensors (pre-permuted / rotation variants)."""
    f32 = np.float32

    def wn(v, g):
        n = np.sqrt((v * v).sum(axis=(1, 2)))
        return (v * (g / n)[:, None, None]).astype(f32)

    w1 = wn(v1, g1)   # (64, 64, 3)
    w2 = wn(v2, g2)   # (32, 64, 3)
    w3 = wn(v3, g3)   # (32, 32, 3)

    # conv lhsT rotation variants.  Ring slot j holds tap k = (j - r) mod 3
    # where r is the rotation (= conv position mod 3).
    def conv_variants(w, nin, nout, nslots):
        # returns (nslots*nin, 3, nout): [slot-block rows, rotation, out]
        out = np.zeros((nslots * nin, 3, nout), f32)
        for r in range(3):
            for j in range(nslots):
                k = (j - r) % 3
                # lhsT rows = input channels of slot j, cols = out channel
                out[j * nin:(j + 1) * nin, r, :] = w[:, :, k].T
        return out

    w1A = conv_variants(w1, 64, 64, 2)            # (128, 3, 64) slots 0,1
    w1C = conv_variants(w1, 64, 64, 3)[128:]      # (64, 3, 64)  slot 2
    w2A = conv_variants(w2, 64, 32, 2)            # (128, 3, 32)
    w2C = conv_variants(w2, 64, 32, 3)[128:]      # (64, 3, 32)
    w3A = conv_variants(w3, 32, 32, 3)            # (96, 3, 32)
    # bias rows: ring tiles carry a constant ones-row as an extra partition,
    # so the conv bias rides in the matmul (lhsT bottom row) and the relus
    # become bias-free single ops placeable on either engine.
    w1C = np.concatenate([w1C, np.tile(b1.reshape(1, 1, 64), (1, 3, 1))], 0)
    w2C = np.concatenate([w2C, np.tile(b2.reshape(1, 1, 32), (1, 3, 1))], 0)
    w3A = np.concatenate([w3A, np.tile(b3.reshape(1, 1, 32), (1, 3, 1))], 0)

    perm = np.array([_perm(r) for r in range(64)])

    # dec = A_mat @ s + Bm_mat @ mx[seg] + c_d   (feedback column, 64-dim)
    W_hpa, W_hpb = W_hp[:, :64], W_hp[:, 64:]
    A_mat = (W_se @ W_hpa).astype(f32)    # (64 emb, 64 feat)
    Bm_mat = (W_se @ W_hpb).astype(f32)
    c_d = (W_se @ b_hp + b_se).astype(f32)
    # doubled vertically so lhsT can be sliced at base partition 0 or 64
    # to match the S/MX band of even/odd steps (matmul requires equal
    # base_partition for lhsT and rhs)
    decA = np.vstack([A_mat[:, perm].T] * 2).copy()   # (128, 64)
    decB = np.vstack([Bm_mat[:, perm].T] * 2).copy()

    # rel endgame: out partition p = 2*k + c (k=step, c=coord).
    # lhsT per slot: (128 rows = [band0: step 2*slot, band1: step 2*slot+1]
    #                 feature rows, 24 cols)
    relA = np.zeros((128, NSLOT, 24), f32)
    relB = np.zeros((128, NSLOT, 24), f32)
    for slot in range(NSLOT):
        for band in range(2):
            k = 2 * slot + band
            rows = slice(band * 64, band * 64 + 64)
            for c in range(2):
                relA[rows, slot, 2 * k + c] = W_hpa[c, perm]
                relB[rows, slot, 2 * k + c] = W_hpb[c, perm]

    bf = ml_dtypes.bfloat16
    return {
        "wse_t": np.concatenate(
            [np.ascontiguousarray(W_se.T, f32), b_se.reshape(1, 64)], 0),
        "w1A": w1A.reshape(128, 3 * 64),
        "w1C": w1C.reshape(65, 3 * 64),
        "w2A": w2A.reshape(128, 3 * 32),
        "w2C": w2C.reshape(65, 3 * 32),
        "w3A": w3A.reshape(97, 3 * 32),
        "decA": decA.astype(bf),
        "decB": decB.astype(bf),
        "relA": relA.reshape(128, NSLOT * 24).astype(bf),
        "relB": relB.reshape(128, NSLOT * 24).astype(bf),
        "b_se": b_se.reshape(64, 1).astype(f32),
        "b_c1": b1.reshape(64, 1).astype(f32),
        "b_c2": b2.reshape(32, 1).astype(f32),
        "b_c3": b3.reshape(32, 1).astype(f32),
        "c_d": c_d.reshape(64, 1),
        "b_hp24": np.tile(b_hp.astype(f32), SEQ).reshape(24, 1),
        "ones": np.ones((1, B), f32),
    }


def _build_module():
    """Build the SPMD Bass module (input-independent, cached)."""
    nc = bacc.Bacc()

    obs_d = nc.dram_tensor("obs", [T, 3, B], F32R, kind="ExternalInput")
    wd = {}
    for name, p, f, dt in [
        ("wse_t", 3, 64, F32R), ("w1A", 128, 192, F32R), ("w1C", 65, 192, F32R),
        ("w2A", 128, 96, F32R), ("w2C", 65, 96, F32R), ("w3A", 97, 96, F32R),
        ("decA", 128, 64, BF16), ("decB", 128, 64, BF16),
        ("relA", 128, NSLOT * 24, BF16), ("relB", 128, NSLOT * 24, BF16),
        ("b_se", 64, 1, F32), ("b_c1", 64, 1, F32), ("b_c2", 32, 1, F32),
        ("b_c3", 32, 1, F32), ("c_d", 64, 1, F32), ("b_hp24", 24, 1, F32),
        ("ones", 1, B, F32R),
    ]:
        wd[name] = nc.dram_tensor(name, [p, f], dt, kind="ExternalInput")
    rels_d = nc.dram_tensor("rels", [24, B], F32, kind="ExternalOutput")

    with tile.TileContext(nc) as tc:
        with (
            tc.tile_pool(name="weights", bufs=1) as wpool,
            tc.tile_pool(name="rings", bufs=1) as rpool,
            tc.tile_pool(name="stage", bufs=3) as xpool,
            tc.tile_pool(name="pdec", bufs=PSUM_BUFS[0], space="PSUM") as pdec,
            tc.tile_pool(name="pc1", bufs=PSUM_BUFS[1], space="PSUM") as pc1,
            tc.tile_pool(name="pc2", bufs=PSUM_BUFS[2], space="PSUM") as pc2,
            tc.tile_pool(name="pc3", bufs=PSUM_BUFS[3], space="PSUM") as pc3,
        ):
            w = {k: wpool.tile_from(v[:], name=k)
                 for k, v in wd.items() if k != "ones"}

            obsA = rpool.tile([128, B], F32R, tag="obsA")   # ring slots 0,1
            obsC = rpool.tile([65, B], F32R, tag="obsC")    # slot 2 + ones row
            c1A = rpool.tile([128, B], F32R, tag="c1A")
            c1C = rpool.tile([65, B], F32R, tag="c1C")
            c2r = rpool.tile([97, B], F32R, tag="c2r")      # 3 bands + ones row
            S_all = rpool.tile([128, NSLOT, B], BF16, tag="S_all")
            MX_all = rpool.tile([128, NSLOT, NS], BF16, tag="MX_all")
            relout = rpool.tile([24, B], F32, tag="relout")

            # memset doesn't support float32r on HW ISA — DMA the ones rows
            nc.sync.dma_start(out=obsC[64:65, :], in_=wd["ones"][:])
            nc.sync.dma_start(out=c1C[64:65, :], in_=wd["ones"][:])
            nc.sync.dma_start(out=c2r[96:97, :], in_=wd["ones"][:])

            def obs_slot(j, ci):
                sl = slice(ci * CH, (ci + 1) * CH)
                if j == 0:
                    return obsA[0:64, sl]
                if j == 1:
                    return obsA[64:128, sl]
                return obsC[0:64, sl]

            def c1_slot(j, ci):
                sl = slice(ci * CH, (ci + 1) * CH)
                if j == 0:
                    return c1A[0:64, sl]
                if j == 1:
                    return c1A[64:128, sl]
                return c1C[0:64, sl]

            for g in range(T + SEQ - 1):           # g = 0..18
                if g < T:
                    xs = xpool.tile([3, B], F32R, tag="xs")
                    nc.sync.dma_start(out=xs[:], in_=obs_d[g])
                # chunk-pair-major emission: each pair runs its whole
                # stage chain before the next pair, tightening the
                # scheduler's cross-engine pipelining
                for cp in range(NCHUNK // 2):
                    for sub in range(2):
                        ci = 2 * cp + sub
                        sl = slice(ci * CH, (ci + 1) * CH)
                        # ---- stage 1: obs column g ----
                        ps = pdec.tile([64, CH], F32, tag="psdec")
                        if g < T:
                            nc.tensor.matmul(ps[:], w["wse_t"][:],
                                             xs[:, sl], start=True, stop=True)
                            nc.vector.tensor_copy(out=obs_slot(g % 3, ci),
                                                  in_=ps[:])
                        else:
                            s = g - T
                            band, slot = (s % 2) * 64, s // 2
                            nc.tensor.matmul(
                                ps[:], w["decA"][band:band + 64, :],
                                S_all[band:band + 64, slot, sl],
                                start=True, stop=False)
                            mxb = (MX_all[band:band + 64, slot,
                                          ci * (CH // SCENE):(ci + 1) * (CH // SCENE)]
                                   .unsqueeze(2).broadcast_to((64, CH // SCENE, SCENE)))
                            nc.tensor.matmul(ps[:], w["decB"][band:band + 64, :],
                                             mxb, start=False, stop=True)
                            nc.vector.tensor_add(obs_slot(g % 3, ci), ps[:],
                                                 w["c_d"][:].broadcast_to((64, CH)))
                        # ---- stage 2: conv1 ----
                        if g >= 2:
                            p = g - 2
                            r = p % 3
                            ps1 = pc1.tile([64, CH], F32, tag="psc1")
                            nc.tensor.matmul(ps1[:],
                                             w["w1A"][:, r * 64:(r + 1) * 64],
                                             obsA[:, sl], start=True, stop=False)
                            nc.tensor.matmul(ps1[:],
                                             w["w1C"][:, r * 64:(r + 1) * 64],
                                             obsC[:, sl], start=False, stop=True)
                            nc.scalar.activation(
                                c1_slot(p % 3, ci), ps1[:],
                                mybir.ActivationFunctionType.Relu)
                    # ---- stage 3: conv2 (wide pair) ----
                    if g >= 4:
                        q = g - 4
                        r = q % 3
                        band = (q % 3) * 32
                        ps2 = pc2.tile([32, 2 * CH], F32, tag="psc2")
                        for sub in range(2):
                            ci = 2 * cp + sub
                            sl = slice(ci * CH, (ci + 1) * CH)
                            half = ps2[:, sub * CH:(sub + 1) * CH]
                            nc.tensor.matmul(half,
                                             w["w2A"][:, r * 32:(r + 1) * 32],
                                             c1A[:, sl], start=True, stop=False)
                            nc.tensor.matmul(half,
                                             w["w2C"][:, r * 32:(r + 1) * 32],
                                             c1C[:, sl], start=False, stop=True)
                        slp = slice(2 * cp * CH, (2 * cp + 2) * CH)
                        nc.scalar.activation(c2r[band:band + 32, slp], ps2[:],
                                             mybir.ActivationFunctionType.Relu)
                    # ---- stage 4+5: conv3 and segmax ----
                    for sub in range(2):
                        ci = 2 * cp + sub
                        sl = slice(ci * CH, (ci + 1) * CH)
                        if g >= 6:
                            u = g - 6
                            r = u % 3
                            ps3 = pc3.tile([32, CH], F32, tag="psc3")
                            nc.tensor.matmul(ps3[:],
                                             w["w3A"][:, r * 32:(r + 1) * 32],
                                             c2r[:, sl], start=True, stop=True)
                            if u <= SEQ - 1:
                                b0 = (u % 2) * 64
                                if ci % 4 == 3:
                                    nc.vector.tensor_scalar_max(
                                        S_all[b0:b0 + 32, u // 2, sl],
                                        ps3[:], 0.0)
                                else:
                                    nc.scalar.activation(
                                        S_all[b0:b0 + 32, u // 2, sl], ps3[:],
                                        mybir.ActivationFunctionType.Relu)
                            if 1 <= u:
                                k = u - 1
                                b1_ = (k % 2) * 64 + 32
                                if u <= SEQ - 1:
                                    nc.vector.tensor_copy(
                                        out=S_all[b1_:b1_ + 32, k // 2, sl],
                                        in_=S_all[(u % 2) * 64:(u % 2) * 64 + 32,
                                                  u // 2, sl])
                                else:
                                    nc.vector.tensor_scalar_max(
                                        S_all[b1_:b1_ + 32, k // 2, sl],
                                        ps3[:], 0.0)
                        if g >= 7:
                            s = g - 7
                            band, slot = (s % 2) * 64, s // 2
                            nc.vector.reduce_max(
                                out=MX_all[band:band + 64, slot,
                                           ci * (CH // SCENE):(ci + 1) * (CH // SCENE)],
                                in_=S_all[band:band + 64, slot, sl]
                                .rearrange("p (s e) -> p s e", e=SCENE),
                                axis=mybir.AxisListType.X)

            # ---- endgame: rels for all 12 steps, M=24 block matmul ----
            if True:
                for ci in range(NCHUNK):
                    sl = slice(ci * CH, (ci + 1) * CH)
                    ps = pdec.tile([24, CH], F32, tag="psdec")
                    for slot in range(NSLOT):
                        nc.tensor.matmul(
                            ps[:], w["relA"][:, slot * 24:(slot + 1) * 24],
                            S_all[:, slot, sl],
                            start=(slot == 0), stop=False)
                        mxb = (MX_all[:, slot,
                                      ci * (CH // SCENE):(ci + 1) * (CH // SCENE)]
                               .unsqueeze(2).broadcast_to((128, CH // SCENE, SCENE)))
                        nc.tensor.matmul(ps[:],
                                         w["relB"][:, slot * 24:(slot + 1) * 24],
                                         mxb, start=False, stop=(slot == NSLOT - 1))
                    nc.scalar.activation(relout[:, sl], ps[:],
                                         mybir.ActivationFunctionType.Identity,
                                         bias=w["b_hp24"][:])
                nc.sync.dma_start(out=rels_d[:], in_=relout[:])

    nc.compile()   # bacc passes: split multi-waits into EventSemaphores etc.
    return nc


def _numpy_fallback(obs_traj, W_se, b_se, v1, g1, b1, v2, g2, b2, v3, g3, b3,
                    W_hp, b_hp, seq_start_end, seq_len):
    """Exact numpy implementation for inputs the device kernel wasn't built
    for (non-uniform segments / different seq_len)."""
    batch = obs_traj.shape[1]
    nseg = seq_start_end.shape[0]
    seg = np.searchsorted(seq_start_end[:, 0], np.arange(batch),
                          side="right") - 1

    def wn(v, g):
        n = np.sqrt((v * v).sum(axis=(1, 2)))
        return v * (g / n)[:, None, None]

    w1, w2, w3 = wn(v1, g1), wn(v2, g2), wn(v3, g3)

    def conv(x, w, b):
        O = w.shape[0]
        Tn = x.shape[2]
        out = np.zeros((x.shape[0], O, Tn - 2), np.float32)
        for t in range(Tn - 2):
            for k in range(3):
                out[:, :, t] += x[:, :, t + k] @ w[:, :, k].T
        return np.maximum(out + b[None, :, None], 0)

    emb = obs_traj @ W_se.T + b_se
    obs_emb = np.transpose(emb, (1, 2, 0)).copy()
    rels = []
    for _ in range(int(seq_len)):
        c3 = conv(conv(conv(obs_emb, w1, b1), w2, b2), w3, b3)
        s = c3.reshape(batch, 64)
        mx = np.full((nseg, 64), -np.inf, np.float32)
        np.maximum.at(mx, seg, s)
        st = np.concatenate([s, mx[seg]], axis=1)
        rel = st @ W_hp.T + b_hp
        dec = rel @ W_se.T + b_se
        obs_emb = np.concatenate([obs_emb[:, :, 1:], dec[:, :, None]], axis=2)
        rels.append(rel)
    return np.stack(rels).astype(np.float32)


def kernel(obs_traj, last_pos, last_pos_rel, W_se, b_se, v1, g1, b1,
           v2, g2, b2, v3, g3, b3, W_hp, b_hp, seq_start_end, seq_len):
    obs_traj = np.asarray(obs_traj, np.float32)
    seq_start_end = np.asarray(seq_start_end)
    args = [np.asarray(a, np.float32) for a in
            (W_se, b_se, v1, g1, b1, v2, g2, b2, v3, g3, b3, W_hp, b_hp)]

    starts = np.arange(BATCH // SCENE, dtype=np.int64) * SCENE
    uniform = (obs_traj.shape == (T, BATCH, 2)
               and int(seq_len) == SEQ
               and seq_start_end.shape == (BATCH // SCENE, 2)
               and np.array_equal(seq_start_end[:, 0], starts)
               and np.array_equal(seq_start_end[:, 1], starts + SCENE))
    if not uniform:
        return _numpy_fallback(obs_traj, *args, seq_start_end, seq_len)

    if "nc" not in _cache:
        _cache["nc"] = _build_module()
    nc = _cache["nc"]

    wdev = _host_weights(*args)
    obs_t = np.concatenate([obs_traj.transpose(0, 2, 1),
                            np.ones((T, 1, BATCH), np.float32)],
                           axis=1)  # (8, 3, 32768) with ones plane

    in_maps = []
    for core in range(NCORES):
        m = dict(wdev)
        m["obs"] = np.ascontiguousarray(obs_t[:, :, core * B:(core + 1) * B])
        in_maps.append(m)

    res = run_bass_kernel_spmd(nc, in_maps, core_ids=list(range(NCORES)))

    out = np.empty((SEQ, BATCH, 2), np.float32)
    for core in range(NCORES):
        arr = res.results[core]["rels"]          # (24, B)
        for c in range(2):
            out[:, core * B:(core + 1) * B, c] = arr[c::2]
    return out



# revision 38
# speedup vs baseline: 1.1222x; 1.1222x over previous
"""Trainium2 Bass kernel for nn_Encoder_66872640799015 (segment_reduce).

Recurrent conv encoder over 32768 pedestrians (4096 scenes x 8), 12 steps.
Sharding: data-parallel over scenes — 8 cores x 4096 pedestrians (512 whole
scenes per core), weights replicated.

Algorithmic structure (v2):
- Rolling ring buffers: each scan step shifts the conv window by one column,
  so only ONE new conv position per layer per step is computed.  Weights are
  pre-permuted per ring rotation; conv biases ride as ones-row lhsT rows so
  every relu is a bias-free single op placeable on any engine.
- Software-pipelined emission: per iteration, stage1 of chunk i is emitted
  alongside conv1 of i-1, conv2 of i-2 and conv3 of i-3, so each
  PE->vector->PE dependency hop has several matmuls of slack and the
  in-order engine queues never block.
- dec's scene-max term is a broadcast-rhs matmul over 65-row MXe/MXo tiles
  (ones row carries c_d), accumulated into the same PSUM as the A-part, so
  the ring write is a single copy; the endgame's MX term is contracted once
  over the 512 distinct scenes and broadcast-added from SBUF.
- Segment-max is a 3-op pairwise bf16 max tree on DVE (2x packed mode) over
  chunk pairs instead of a full-rate tensor_reduce.
- TRN2 legality: GpSimd(Pool) touches no PSUM and runs no TensorTensor;
  no DVE broadcast operand reads PSUM (scene terms are staged via SBUF).
"""

import sys

sys.path.insert(0, "/opt/trn_rl_repo")

import numpy as np
import ml_dtypes

import concourse.bass as bass
import concourse.bacc as bacc
import concourse.tile as tile
from concourse import mybir
from concourse.bass_utils import run_bass_kernel_spmd

NCORES = 8
BATCH = 32768
B = BATCH // NCORES        # pedestrians per core
T = 8                      # obs_len
SEQ = 12                   # seq_len
SCENE = 8                  # pedestrians per scene
NS = B // SCENE            # scenes per core (512)
CH = 512                   # free-dim chunk (one PSUM bank of fp32)
NCHUNK = B // CH
NSLOT = SEQ // 2           # S_all free slots (2 steps per slot)

F32 = mybir.dt.float32
F32R = mybir.dt.float32r
BF16 = mybir.dt.bfloat16

_cache = {}

# engine-assignment knobs: which engine runs each elementwise op
# 'v' = DVE (vector), 'a' = ACT (scalar), 'p' = Pool (gpsimd)
KNOBS = dict(
    embed='v',       # engine rotation for embed-phase psum->ring copies
    dec_add='v',     # engine rotation for the dec psum->ring copies
    combine='v',
    r1='a', r2='a', r3='av',
    hi='v',
    seg='tree',      # 'reduce' | 'tree'
    seg_eng='p',     # engine for the max tree (SBUF-only op)
    psum=(2, 1, 2, 1, 1),   # (pdec, psB, pc1, pc2-wide, pc3)
    borrow=True,     # embed borrows idle conv psum pools during fill
    special7=False,   # split max-tree for chunk 7 (psB chain shortening)
)


def _perm(r):
    """S-feature row (32*t + ch) -> reference feature index (2*ch + t)."""
    t, ch = r // 32, r % 32
    return 2 * ch + t


def _host_weights(W_se, b_se, v1, g1, b1, v2, g2, b2, v3, g3, b3, W_hp, b_hp):
    """Derive all device weight tensors (pre-permuted / rotation variants)."""
    f32 = np.float32

    def wn(v, g):
        n = np.sqrt((v * v).sum(axis=(1, 2)))
        return (v * (g / n)[:, None, None]).astype(f32)

    w1 = wn(v1, g1)   # (64, 64, 3)
    w2 = wn(v2, g2)   # (32, 64, 3)
    w3 = wn(v3, g3)   # (32, 32, 3)

    # conv lhsT rotation variants.  Ring slot j holds tap k = (j - r) mod 3
    # where r is the rotation (= conv position mod 3).
    def conv_variants(w, nin, nout, nslots):
        # returns (nslots*nin, 3, nout): [slot-block rows, rotation, out]
        out = np.zeros((nslots * nin, 3, nout), f32)
        for r in range(3):
            for j in range(nslots):
                k = (j - r) % 3
                # lhsT rows = input channels of slot j, cols = out channel
                out[j * nin:(j + 1) * nin, r, :] = w[:, :, k].T
        return out

    w1A = conv_variants(w1, 64, 64, 2)            # (128, 3, 64) slots 0,1
    w1C = conv_variants(w1, 64, 64, 3)[128:]      # (64, 3, 64)  slot 2
    w2A = conv_variants(w2, 64, 32, 2)            # (128, 3, 32)
    w2C = conv_variants(w2, 64, 32, 3)[128:]      # (64, 3, 32)
    w3A = conv_variants(w3, 32, 32, 3)            # (96, 3, 32)
    # bias rows: ring tiles carry a constant ones-row as an extra partition,
    # so the conv bias rides in the matmul (lhsT bottom row) and the relus
    # become bias-free single ops placeable on any engine.
    w1C = np.concatenate([w1C, np.tile(b1.reshape(1, 1, 64), (1, 3, 1))], 0)
    w2C = np.concatenate([w2C, np.tile(b2.reshape(1, 1, 32), (1, 3, 1))], 0)
    w3A = np.concatenate([w3A, np.tile(b3.reshape(1, 1, 32), (1, 3, 1))], 0)

    perm = np.array([_perm(r) for r in range(64)])

    # dec = A_mat @ s + Bm_mat @ mx[seg] + c_d   (feedback column, 64-dim)
    # rel = W_hpa @ s + W_hpb @ mx[seg] + b_hp   (2-dim, rides as 2 extra
    # output columns on the same matmuls)
    W_hpa, W_hpb = W_hp[:, :64], W_hp[:, 64:]
    A_mat = (W_se @ W_hpa).astype(f32)    # (64 emb, 64 feat)
    Bm_mat = (W_se @ W_hpb).astype(f32)
    c_d = (W_se @ b_hp + b_se).astype(f32)

    # decA: (128, 64) doubled vertically so lhsT can be sliced at base
    # partition 0 or 64 to match the S band of even/odd steps.
    decA = np.vstack([A_mat[:, perm].T] * 2).copy()        # (128, 64)

    # decB65: (65, 64): rows 0:64 contract the 64 MX rows, row 64 is the
    # ones row -> carries c_d.
    decB65 = np.zeros((65, 64), f32)
    decB65[:64, :] = Bm_mat[:, perm].T
    decB65[64, :] = c_d

    # rel endgame: out partition p = 2*k + c (k=step, c=coord).
    # relA lhsT per slot: (128 rows = [band0: step 2*slot, band1: step
    # 2*slot+1] feature rows, 24 cols).  relBe/relBo contract the 65-row
    # MXe/MXo tiles per slot; b_hp rides the ones row of relBe slot 0.
    relA = np.zeros((128, NSLOT, 24), f32)
    relBe = np.zeros((65, NSLOT, 24), f32)
    relBo = np.zeros((65, NSLOT, 24), f32)
    for slot in range(NSLOT):
        for band in range(2):
            k = 2 * slot + band
            rows = slice(band * 64, band * 64 + 64)
            dst = relBe if band == 0 else relBo
            for c in range(2):
                relA[rows, slot, 2 * k + c] = W_hpa[c, perm]
                dst[:64, slot, 2 * k + c] = W_hpb[c, perm]
    relBe[64, 0, :] = np.tile(b_hp, SEQ)

    bf = ml_dtypes.bfloat16
    return {
        "wse_t": np.concatenate(
            [np.ascontiguousarray(W_se.T, f32), b_se.reshape(1, 64)], 0),
        "w1A": w1A.reshape(128, 3 * 64),
        "w1C": w1C.reshape(65, 3 * 64),
        "w2A": w2A.reshape(128, 3 * 32),
        "w2C": w2C.reshape(65, 3 * 32),
        "w3A": w3A.reshape(97, 3 * 32),
        "decA": decA.astype(bf),
        "decB65": decB65.astype(bf),
        "relA": relA.reshape(128, NSLOT * 24).astype(bf),
        "relBe": relBe.reshape(65, NSLOT * 24).astype(bf),
        "relBo": relBo.reshape(65, NSLOT * 24).astype(bf),
        "ones": np.ones((1, B), np.float32),
        "ones_bf": np.ones((1, 2 * NSLOT * NS), bf),
    }


def _build_module():
    """Build the SPMD Bass module (input-independent, cached)."""
    nc = bacc.Bacc()

    obs_d = nc.dram_tensor("obs", [T, 3, B], F32R, kind="ExternalInput")
    wd = {}
    for name, p, f, dt in [
        ("wse_t", 3, 64, F32R), ("w1A", 128, 192, F32R), ("w1C", 65, 192, F32R),
        ("w2A", 128, 96, F32R), ("w2C", 65, 96, F32R), ("w3A", 97, 96, F32R),
        ("decA", 128, 64, BF16), ("decB65", 65, 64, BF16),
        ("relA", 128, NSLOT * 24, BF16), ("relBe", 65, NSLOT * 24, BF16),
        ("relBo", 65, NSLOT * 24, BF16),
        ("ones", 1, B, F32R), ("ones_bf", 1, 2 * NSLOT * NS, BF16),
    ]:
        wd[name] = nc.dram_tensor(name, [p, f], dt, kind="ExternalInput")
    rels_d = nc.dram_tensor("rels", [24, B], F32, kind="ExternalOutput")

    with tile.TileContext(nc) as tc:
        with (
            tc.tile_pool(name="weights", bufs=1) as wpool,
            tc.tile_pool(name="rings", bufs=1) as rpool,
            tc.tile_pool(name="stage", bufs=2) as xpool,
            tc.tile_pool(name="pdec", bufs=KNOBS['psum'][0], space="PSUM") as pdec,
            tc.tile_pool(name="psB", bufs=KNOBS['psum'][1], space="PSUM") as psBp,
            tc.tile_pool(name="pc1", bufs=KNOBS['psum'][2], space="PSUM") as pc1,
            tc.tile_pool(name="pc2", bufs=KNOBS['psum'][3], space="PSUM") as pc2,
            tc.tile_pool(name="pc3", bufs=KNOBS['psum'][4], space="PSUM") as pc3,
        ):
            # DMA issue order = HWDGE service order: load the tensors the
            # pipeline needs first (obs column 0, embed + conv1 weights)
            xs = [None, None]
            xs[0] = xpool.tile([3, B], F32R, tag="xs", name="xs")
            nc.sync.dma_start(out=xs[0][:], in_=obs_d[0])
            w = {}
            for k in ("wse_t", "w1A", "w1C"):
                w[k] = wpool.tile_from(wd[k][:], name=k)
            xs[1] = xpool.tile([3, B], F32R, tag="xs", name="xs")
            nc.sync.dma_start(out=xs[1][:], in_=obs_d[1])
            for k, v in wd.items():
                if k not in ("ones", "ones_bf") and k not in w:
                    w[k] = wpool.tile_from(v[:], name=k)

            obsA = rpool.tile([128, B], F32R, tag="obsA")   # ring slots 0,1
            obsC = rpool.tile([65, B], F32R, tag="obsC")    # slot 2 + ones row
            c1A = rpool.tile([128, B], F32R, tag="c1A")
            c1C = rpool.tile([65, B], F32R, tag="c1C")
            c2r = rpool.tile([97, B], F32R, tag="c2r")      # 3 bands + ones row
            S_all = rpool.tile([128, NSLOT, B], BF16, tag="S_all")
            # MX split even/odd so each tile has a ones row (row 64) that
            # carries c_d / b_hp through the decB66 matmul.
            MXe = rpool.tile([65, NSLOT, NS], BF16, tag="MXe")
            MXo = rpool.tile([65, NSLOT, NS], BF16, tag="MXo")
            relout = rpool.tile([24, B], F32, tag="relout")
            st1 = [rpool.tile([64, NS // 4, 4], BF16, tag=f"st1_{i}",
                              name=f"st1_{i}") for i in range(2)]
            st2 = [rpool.tile([64, NS // 4, 2], BF16, tag=f"st2_{i}",
                              name=f"st2_{i}") for i in range(2)]

            # memset doesn't support float32r on HW ISA — DMA the ones rows
            nc.sync.dma_start(out=obsC[64:65, :], in_=wd["ones"][:])
            nc.sync.dma_start(out=c1C[64:65, :], in_=wd["ones"][:])
            nc.sync.dma_start(out=c2r[96:97, :], in_=wd["ones"][:])
            nc.sync.dma_start(
                out=MXe[64:65, :, :],
                in_=wd["ones_bf"][:, :NSLOT * NS].rearrange(
                    "o (s n) -> o s n", s=NSLOT))
            nc.sync.dma_start(
                out=MXo[64:65, :, :],
                in_=wd["ones_bf"][:, NSLOT * NS:].rearrange(
                    "o (s n) -> o s n", s=NSLOT))

            def eng(sel):
                return {'v': nc.vector, 'a': nc.scalar, 'p': nc.gpsimd}[sel]

            def copy_op(sel, out, in_):
                if sel == 'a':
                    nc.scalar.activation(
                        out, in_, mybir.ActivationFunctionType.Identity)
                else:
                    eng(sel).tensor_copy(out=out, in_=in_)

            def relu_op(sel, out, in_):
                if sel == 'a':
                    nc.scalar.activation(
                        out, in_, mybir.ActivationFunctionType.Relu)
                else:
                    eng(sel).tensor_scalar_max(out, in_, 0.0)

            def obs_slot(j, ci):
                sl = slice(ci * CH, (ci + 1) * CH)
                if j == 0:
                    return obsA[0:64, sl]
                if j == 1:
                    return obsA[64:128, sl]
                return obsC[0:64, sl]

            def c1_slot(j, ci):
                sl = slice(ci * CH, (ci + 1) * CH)
                if j == 0:
                    return c1A[0:64, sl]
                if j == 1:
                    return c1A[64:128, sl]
                return c1C[0:64, sl]

            def mx_tile(s):
                return MXe if s % 2 == 0 else MXo

            sbRelMx = rpool.tile([24, NS], F32, tag="sbRelMx")

            def stage1(g, ci):
                sl = slice(ci * CH, (ci + 1) * CH)
                ssl = slice(ci * (CH // SCENE), (ci + 1) * (CH // SCENE))
                if g < T:
                    if KNOBS['borrow'] and g < 4:
                        pool_rr = [pdec, pc2, pc3, psBp][ci % 4]
                        tag_rr = ["psdec", "psc2", "psc3", "psB"][ci % 4]
                    elif KNOBS['borrow'] and g < 6:
                        pool_rr = [pdec, pc3, psBp][ci % 3]
                        tag_rr = ["psdec", "psc3", "psB"][ci % 3]
                    elif KNOBS['borrow']:
                        pool_rr = [pdec, psBp][ci % 2]
                        tag_rr = ["psdec", "psB"][ci % 2]
                    else:
                        pool_rr, tag_rr = pdec, "psdec"
                    ps = pool_rr.tile([64, CH], F32, tag=tag_rr, name="psE")
                    nc.tensor.matmul(ps[:], w["wse_t"][:],
                                     xs[g % 2][:, sl], start=True, stop=True)
                    copy_op(KNOBS['embed'][ci % len(KNOBS['embed'])], obs_slot(g % 3, ci), ps[:])
                else:
                    s = g - T
                    band = (s % 2) * 64
                    slot = s // 2
                    ps = pdec.tile([64, CH], F32, tag="psdec", name="psA")
                    nc.tensor.matmul(
                        ps[:], w["decA"][band:band + 64, :],
                        S_all[band:band + 64, slot, sl],
                        start=True, stop=False)
                    mxb = (mx_tile(s)[:, slot, ssl].unsqueeze(2)
                           .broadcast_to((65, CH // SCENE, SCENE)))
                    nc.tensor.matmul(ps[:], w["decB65"][:], mxb,
                                     start=False, stop=True)
                    da = KNOBS['dec_add']
                    copy_op(da[ci % len(da)], obs_slot(g % 3, ci), ps[:])

            def conv1(g, ci):
                sl = slice(ci * CH, (ci + 1) * CH)
                p = g - 2
                r = p % 3
                ps1 = pc1.tile([64, CH], F32, tag="psc1")
                # read the stale ring tile first: the tile holding this
                # iteration's fresh column gates on the stage-1 add
                mmA = (w["w1A"][:, r * 64:(r + 1) * 64], obsA[:, sl])
                mmC = (w["w1C"][:, r * 64:(r + 1) * 64], obsC[:, sl])
                first, second = (mmC, mmA) if g % 3 != 2 else (mmA, mmC)
                nc.tensor.matmul(ps1[:], first[0], first[1],
                                 start=True, stop=False)
                nc.tensor.matmul(ps1[:], second[0], second[1],
                                 start=False, stop=True)
                r = KNOBS['r1']
                relu_op(r[ci % len(r)], c1_slot(p % 3, ci), ps1[:])

            ps2_pair = [None]

            def conv2(g, ci):
                sl = slice(ci * CH, (ci + 1) * CH)
                q = g - 4
                r = q % 3
                band2 = (q % 3) * 32
                if ci % 2 == 0:
                    ps2_pair[0] = pc2.tile([32, 2 * CH], F32, tag="psc2",
                                           name="ps2")
                half = ps2_pair[0][:, (ci % 2) * CH:(ci % 2) * CH + CH]
                nc.tensor.matmul(half,
                                 w["w2A"][:, r * 32:(r + 1) * 32],
                                 c1A[:, sl], start=True, stop=False)
                nc.tensor.matmul(half,
                                 w["w2C"][:, r * 32:(r + 1) * 32],
                                 c1C[:, sl], start=False, stop=True)
                if ci % 2 == 1:
                    slp = slice((ci - 1) * CH, (ci + 1) * CH)
                    r = KNOBS['r2']
                    relu_op(r[(ci // 2) % len(r)],
                            c2r[band2:band2 + 32, slp], ps2_pair[0][:])

            ps3_last = [None]

            def conv3_mm(g, ci):
                sl = slice(ci * CH, (ci + 1) * CH)
                u = g - 6
                r = u % 3
                ps3_last[0] = pc3.tile([32, CH], F32, tag="psc3", name="ps3")
                ps3 = ps3_last[0]
                nc.tensor.matmul(ps3[:],
                                 w["w3A"][:, r * 32:(r + 1) * 32],
                                 c2r[:, sl], start=True, stop=True)
                if u <= SEQ - 1:
                    b0 = (u % 2) * 64
                    r = KNOBS['r3']
                    relu_op(r[ci % len(r)], S_all[b0:b0 + 32, u // 2, sl],
                            ps3[:])

            def conv3_post(g, ci):
                # pair-granular (runs at odd ci over chunks ci-1, ci): wide
                # hi copy on DVE, wide max-tree on the knob engine —
                # halves per-op overhead and keeps Pool SBUF-only
                if ci % 2 == 0:
                    return
                slp = slice((ci - 1) * CH, (ci + 1) * CH)
                sslp = slice((ci - 1) * (CH // SCENE),
                             (ci + 1) * (CH // SCENE))
                u = g - 6
                if 1 <= u:
                    k = u - 1
                    b1_ = (k % 2) * 64 + 32
                    if u <= SEQ - 1:
                        copy_op(KNOBS['hi'],
                                S_all[b1_:b1_ + 32, k // 2, slp],
                                S_all[(u % 2) * 64:(u % 2) * 64 + 32,
                                      u // 2, slp])
                    else:
                        for cj in (ci - 1, ci):
                            relu_op('v',
                                    S_all[b1_:b1_ + 32, k // 2,
                                          cj * CH:(cj + 1) * CH],
                                    ps3_cache[cj][:])
                if g < 7:
                    return
                s2 = g - 7
                band = (s2 % 2) * 64
                sv = (S_all[band:band + 64, s2 // 2, slp]
                      .rearrange("p (s e) -> p s e", e=SCENE))
                if KNOBS['seg'] == 'tree':
                    se = eng(KNOBS['seg_eng'])
                    t1, t2 = st1[(ci // 2) % 2], st2[(ci // 2) % 2]
                    se.tensor_max(t1[:], sv[:, :, 0:4], sv[:, :, 4:8])
                    se.tensor_max(t2[:], t1[:, :, 0:2], t1[:, :, 2:4])
                    se.tensor_max(
                        mx_tile(s2)[0:64, s2 // 2, sslp],
                        t2[:, :, 0], t2[:, :, 1])
                else:
                    nc.vector.reduce_max(
                        out=mx_tile(s2)[0:64, s2 // 2, sslp],
                        in_=sv, axis=mybir.AxisListType.X)

            # software-pipelined emission: stage1 of chunk i is emitted
            # alongside conv1 of i-1, conv2 of i-2, conv3 of i-3 so every
            # PE->vector->PE hop has several matmuls of slack
            ps3_cache = [None] * NCHUNK
            for g in range(T + SEQ - 1):           # g = 0..18
                for si in range(NCHUNK + 3):
                    if si < NCHUNK:
                        stage1(g, si)
                    if g >= 2 and 1 <= si <= NCHUNK:
                        conv1(g, si - 1)
                    if g >= 4 and 2 <= si <= NCHUNK + 1:
                        conv2(g, si - 2)
                    if g >= 6 and 3 <= si <= NCHUNK + 2:
                        conv3_mm(g, si - 3)
                        ps3_cache[si - 3] = ps3_last[0]
                        conv3_post(g, si - 3)
                if g + 2 < T:
                    xs[g % 2] = xpool.tile([3, B], F32R, tag="xs", name="xs")
                    nc.sync.dma_start(out=xs[g % 2][:], in_=obs_d[g + 2])
                # ---- shared per-step scene contribution for step s=g-7 ----

            # ---- endgame: rels for all 12 steps ----
            # shared scene contribution: psRelMx (24, NS) accumulated over
            # all 12 half-slots (b_hp rides relBe slot 0's ones row)
            psRelMx = psBp.tile([24, CH], F32, tag="psB")
            for slot in range(NSLOT):
                nc.tensor.matmul(psRelMx[:],
                                 w["relBe"][:, slot * 24:(slot + 1) * 24],
                                 MXe[:, slot, :],
                                 start=(slot == 0), stop=False)
                nc.tensor.matmul(psRelMx[:],
                                 w["relBo"][:, slot * 24:(slot + 1) * 24],
                                 MXo[:, slot, :],
                                 start=False, stop=(slot == NSLOT - 1))
            nc.vector.tensor_copy(out=sbRelMx[:], in_=psRelMx[:])
            for ci in range(NCHUNK):
                sl = slice(ci * CH, (ci + 1) * CH)
                ssl = slice(ci * (CH // SCENE), (ci + 1) * (CH // SCENE))
                ps = pdec.tile([24, CH], F32, tag="psdec", name="psR")
                for slot in range(NSLOT):
                    nc.tensor.matmul(
                        ps[:], w["relA"][:, slot * 24:(slot + 1) * 24],
                        S_all[:, slot, sl],
                        start=(slot == 0), stop=(slot == NSLOT - 1))
                rmx = (sbRelMx[:, ssl].unsqueeze(2)
                       .broadcast_to((24, CH // SCENE, SCENE)))
                cmb = KNOBS['combine']
                eng(cmb[ci % len(cmb)]).tensor_add(relout[:, sl], ps[:], rmx)
                nc.sync.dma_start(out=rels_d[:, sl], in_=relout[:, sl])

    nc.compile()   # bacc passes: split multi-waits into EventSemaphores etc.
    return nc


def _numpy_fallback(obs_traj, W_se, b_se, v1, g1, b1, v2, g2, b2, v3, g3, b3,
                    W_hp, b_hp, seq_start_end, seq_len):
    """Exact numpy implementation for inputs the device kernel wasn't built
    for (non-uniform segments / different seq_len)."""
    batch = obs_traj.shape[1]
    nseg = seq_start_end.shape[0]
    seg = np.searchsorted(seq_start_end[:, 0], np.arange(batch),
                          side="right") - 1

    def wn(v, g):
        n = np.sqrt((v * v).sum(axis=(1, 2)))
        return v * (g / n)[:, None, None]

    w1, w2, w3 = wn(v1, g1), wn(v2, g2), wn(v3, g3)

    def conv(x, w, b):
        O = w.shape[0]
        Tn = x.shape[2]
        out = np.zeros((x.shape[0], O, Tn - 2), np.float32)
        for t in range(Tn - 2):
            for k in range(3):
                out[:, :, t] += x[:, :, t + k] @ w[:, :, k].T
        return np.maximum(out + b[None, :, None], 0)

    emb = obs_traj @ W_se.T + b_se
    obs_emb = np.transpose(emb, (1, 2, 0)).copy()
    rels = []
    for _ in range(int(seq_len)):
        c3 = conv(conv(conv(obs_emb, w1, b1), w2, b2), w3, b3)
        s = c3.reshape(batch, 64)
        mx = np.full((nseg, 64), -np.inf, np.float32)
        np.maximum.at(mx, seg, s)
        st = np.concatenate([s, mx[seg]], axis=1)
        rel = st @ W_hp.T + b_hp
        dec = rel @ W_se.T + b_se
        obs_emb = np.concatenate([obs_emb[:, :, 1:], dec[:, :, None]], axis=2)
        rels.append(rel)
    return np.stack(rels).astype(np.float32)


def kernel(obs_traj, last_pos, last_pos_rel, W_se, b_se, v1, g1, b1,
           v2, g2, b2, v3, g3, b3, W_hp, b_hp, seq_start_end, seq_len):
    obs_traj = np.asarray(obs_traj, np.float32)
    seq_start_end = np.asarray(seq_start_end)
    args = [np.asarray(a, np.float32) for a in
            (W_se, b_se, v1, g1, b1, v2, g2, b2, v3, g3, b3, W_hp, b_hp)]

    starts = np.arange(BATCH // SCENE, dtype=np.int64) * SCENE
    uniform = (obs_traj.shape == (T, BATCH, 2)
               and int(seq_len) == SEQ
               and seq_start_end.shape == (BATCH // SCENE, 2)
               and np.array_equal(seq_start_end[:, 0], starts)
               and np.array_equal(seq_start_end[:, 1], starts + SCENE))
    if not uniform:
        return _numpy_fallback(obs_traj, *args, seq_start_end, seq_len)

    if "nc" not in _cache:
        _cache["nc"] = _build_module()
    nc = _cache["nc"]

    wdev = _host_weights(*args)
    obs_t = np.concatenate([obs_traj.transpose(0, 2, 1),
                            np.ones((T, 1, BATCH), np.float32)],
                           axis=1)  # (8, 3, 32768) with ones plane

    in_maps = []
    for core in range(NCORES):
        m = dict(wdev)
        m["obs"] = np.ascontiguousarray(obs_t[:, :, core * B:(core + 1) * B])
        in_maps.append(m)

    res = run_bass_kernel_spmd(nc, in_maps, core_ids=list(range(NCORES)))

    out = np.empty((SEQ, BATCH, 2), np.float32)
    for core in range(NCORES):
        arr = res.results[core]["rels"]          # (24, B)
        for c in range(2):
            out[:, core * B:(core + 1) * B, c] = arr[c::2]
    return out


# revision 42
# speedup vs baseline: 1.1407x; 1.0165x over previous
"""Trainium2 Bass kernel for nn_Encoder_66872640799015 (segment_reduce).

Recurrent conv encoder over 32768 pedestrians (4096 scenes x 8), 12 steps.
Sharding: data-parallel over scenes — 8 cores x 4096 pedestrians (512 whole
scenes per core), weights replicated.

Algorithmic structure (v2):
- Rolling ring buffers: each scan step shifts the conv window by one column,
  so only ONE new conv position per layer per step is computed.  Weights are
  pre-permuted per ring rotation; conv biases ride as ones-row lhsT rows so
  every relu is a bias-free single op placeable on any engine.
- Software-pipelined emission: per iteration, stage1 of chunk i is emitted
  alongside conv1 of i-1, conv2 of i-2 and conv3 of i-3, so each
  PE->vector->PE dependency hop has several matmuls of slack and the
  in-order engine queues never block.
- dec's scene-max term is a broadcast-rhs matmul over 65-row MXe/MXo tiles
  (ones row carries c_d), accumulated into the same PSUM as the A-part, so
  the ring write is a single copy; the endgame's MX term is contracted once
  over the 512 distinct scenes and broadcast-added from SBUF.
- Segment-max is a 3-op pairwise bf16 max tree on DVE (2x packed mode) over
  chunk pairs instead of a full-rate tensor_reduce.
- TRN2 legality: GpSimd(Pool) touches no PSUM and runs no TensorTensor;
  no DVE broadcast operand reads PSUM (scene terms are staged via SBUF).
"""

import sys

sys.path.insert(0, "/opt/trn_rl_repo")

import numpy as np
import ml_dtypes

import concourse.bass as bass
import concourse.bacc as bacc
import concourse.tile as tile
from concourse import mybir
from concourse.bass_utils import run_bass_kernel_spmd

NCORES = 8
BATCH = 32768
B = BATCH // NCORES        # pedestrians per core
T = 8                      # obs_len
SEQ = 12                   # seq_len
SCENE = 8                  # pedestrians per scene
NS = B // SCENE            # scenes per core (512)
CH = 512                   # free-dim chunk (one PSUM bank of fp32)
NCHUNK = B // CH
NSLOT = SEQ // 2           # S_all free slots (2 steps per slot)

F32 = mybir.dt.float32
F32R = mybir.dt.float32r
BF16 = mybir.dt.bfloat16

_cache = {}

# engine-assignment knobs: which engine runs each elementwise op
# 'v' = DVE (vector), 'a' = ACT (scalar), 'p' = Pool (gpsimd)
KNOBS = dict(
    embed='v',       # engine rotation for embed-phase psum->ring copies
    dec_add='v',     # engine rotation for the dec psum->ring copies
    combine='v',
    r1='a', r2='a', r3='av',
    hi='v',
    seg='tree',      # 'reduce' | 'tree'
    seg_eng='p',     # engine for the max tree (SBUF-only op)
    psum=(2, 1, 2, 1, 1),   # (pdec, psB, pc1, pc2-wide, pc3)
    borrow=True,     # embed borrows idle conv psum pools during fill
    special7=False,   # split max-tree for chunk 7 (psB chain shortening)
)


def _perm(r):
    """S-feature row (32*t + ch) -> reference feature index (2*ch + t)."""
    t, ch = r // 32, r % 32
    return 2 * ch + t


def _host_weights(W_se, b_se, v1, g1, b1, v2, g2, b2, v3, g3, b3, W_hp, b_hp):
    """Derive all device weight tensors (pre-permuted / rotation variants)."""
    f32 = np.float32

    def wn(v, g):
        n = np.sqrt((v * v).sum(axis=(1, 2)))
        return (v * (g / n)[:, None, None]).astype(f32)

    w1 = wn(v1, g1)   # (64, 64, 3)
    w2 = wn(v2, g2)   # (32, 64, 3)
    w3 = wn(v3, g3)   # (32, 32, 3)

    # conv lhsT rotation variants.  Ring slot j holds tap k = (j - r) mod 3
    # where r is the rotation (= conv position mod 3).
    def conv_variants(w, nin, nout, nslots):
        # returns (nslots*nin, 3, nout): [slot-block rows, rotation, out]
        out = np.zeros((nslots * nin, 3, nout), f32)
        for r in range(3):
            for j in range(nslots):
                k = (j - r) % 3
                # lhsT rows = input channels of slot j, cols = out channel
                out[j * nin:(j + 1) * nin, r, :] = w[:, :, k].T
        return out

    w1A = conv_variants(w1, 64, 64, 2)            # (128, 3, 64) slots 0,1
    w1C = conv_variants(w1, 64, 64, 3)[128:]      # (64, 3, 64)  slot 2
    w2A = conv_variants(w2, 64, 32, 2)            # (128, 3, 32)
    w2C = conv_variants(w2, 64, 32, 3)[128:]      # (64, 3, 32)
    w3A = conv_variants(w3, 32, 32, 3)            # (96, 3, 32)
    # bias rows: ring tiles carry a constant ones-row as an extra partition,
    # so the conv bias rides in the matmul (lhsT bottom row) and the relus
    # become bias-free single ops placeable on any engine.
    w1C = np.concatenate([w1C, np.tile(b1.reshape(1, 1, 64), (1, 3, 1))], 0)
    w2C = np.concatenate([w2C, np.tile(b2.reshape(1, 1, 32), (1, 3, 1))], 0)
    w3A = np.concatenate([w3A, np.tile(b3.reshape(1, 1, 32), (1, 3, 1))], 0)

    perm = np.array([_perm(r) for r in range(64)])

    # dec = A_mat @ s + Bm_mat @ mx[seg] + c_d   (feedback column, 64-dim)
    # rel = W_hpa @ s + W_hpb @ mx[seg] + b_hp   (2-dim, rides as 2 extra
    # output columns on the same matmuls)
    W_hpa, W_hpb = W_hp[:, :64], W_hp[:, 64:]
    A_mat = (W_se @ W_hpa).astype(f32)    # (64 emb, 64 feat)
    Bm_mat = (W_se @ W_hpb).astype(f32)
    c_d = (W_se @ b_hp + b_se).astype(f32)

    # decA: (128, 64) doubled vertically so lhsT can be sliced at base
    # partition 0 or 64 to match the S band of even/odd steps.
    decA = np.vstack([A_mat[:, perm].T] * 2).copy()        # (128, 64)

    # decB65: (65, 64): rows 0:64 contract the 64 MX rows, row 64 is the
    # ones row -> carries c_d.
    decB65 = np.zeros((65, 64), f32)
    decB65[:64, :] = Bm_mat[:, perm].T
    decB65[64, :] = c_d

    # rel endgame: out partition p = 2*k + c (k=step, c=coord).
    # relA lhsT per slot: (128 rows = [band0: step 2*slot, band1: step
    # 2*slot+1] feature rows, 24 cols).  relBe/relBo contract the 65-row
    # MXe/MXo tiles per slot; b_hp rides the ones row of relBe slot 0.
    relA = np.zeros((128, NSLOT, 24), f32)
    relBe = np.zeros((65, NSLOT, 24), f32)
    relBo = np.zeros((65, NSLOT, 24), f32)
    for slot in range(NSLOT):
        for band in range(2):
            k = 2 * slot + band
            rows = slice(band * 64, band * 64 + 64)
            dst = relBe if band == 0 else relBo
            for c in range(2):
                relA[rows, slot, 2 * k + c] = W_hpa[c, perm]
                dst[:64, slot, 2 * k + c] = W_hpb[c, perm]
    relBe[64, 0, :] = np.tile(b_hp, SEQ)

    bf = ml_dtypes.bfloat16
    return {
        "wse_t": np.concatenate(
            [np.ascontiguousarray(W_se.T, f32), b_se.reshape(1, 64)], 0),
        "w1A": w1A.reshape(128, 3 * 64),
        "w1C": w1C.reshape(65, 3 * 64),
        "w2A": w2A.reshape(128, 3 * 32),
        "w2C": w2C.reshape(65, 3 * 32),
        "w3A": w3A.reshape(97, 3 * 32),
        "decA": decA.astype(bf),
        "decB65": decB65.astype(bf),
        "relA": relA.reshape(128, NSLOT * 24).astype(bf),
        "relBe": relBe.reshape(65, NSLOT * 24).astype(bf),
        "relBo": relBo.reshape(65, NSLOT * 24).astype(bf),
        "ones": np.ones((1, B), np.float32),
        "ones_bf": np.ones((1, 2 * NSLOT * NS), bf),
    }


def _build_module():
    """Build the SPMD Bass module (input-independent, cached)."""
    nc = bacc.Bacc()

    obs_d = nc.dram_tensor("obs", [T, 3, B], F32R, kind="ExternalInput")
    wd = {}
    for name, p, f, dt in [
        ("wse_t", 3, 64, F32R), ("w1A", 128, 192, F32R), ("w1C", 65, 192, F32R),
        ("w2A", 128, 96, F32R), ("w2C", 65, 96, F32R), ("w3A", 97, 96, F32R),
        ("decA", 128, 64, BF16), ("decB65", 65, 64, BF16),
        ("relA", 128, NSLOT * 24, BF16), ("relBe", 65, NSLOT * 24, BF16),
        ("relBo", 65, NSLOT * 24, BF16),
        ("ones", 1, B, F32R), ("ones_bf", 1, 2 * NSLOT * NS, BF16),
    ]:
        wd[name] = nc.dram_tensor(name, [p, f], dt, kind="ExternalInput")
    rels_d = nc.dram_tensor("rels", [24, B], F32, kind="ExternalOutput")

    with tile.TileContext(nc) as tc:
        with (
            tc.tile_pool(name="weights", bufs=1) as wpool,
            tc.tile_pool(name="rings", bufs=1) as rpool,
            tc.tile_pool(name="stage", bufs=2) as xpool,
            tc.tile_pool(name="pdec", bufs=KNOBS['psum'][0], space="PSUM") as pdec,
            tc.tile_pool(name="psB", bufs=KNOBS['psum'][1], space="PSUM") as psBp,
            tc.tile_pool(name="pc1", bufs=KNOBS['psum'][2], space="PSUM") as pc1,
            tc.tile_pool(name="pc2", bufs=KNOBS['psum'][3], space="PSUM") as pc2,
            tc.tile_pool(name="pc3", bufs=KNOBS['psum'][4], space="PSUM") as pc3,
        ):
            # DMA issue order = HWDGE service order: load the tensors the
            # pipeline needs first (obs column 0, embed + conv1 weights)
            xs = [None, None]
            xs[0] = xpool.tile([3, B], F32R, tag="xs", name="xs")
            nc.sync.dma_start(out=xs[0][:], in_=obs_d[0])
            w = {}
            for k in ("wse_t", "w1A", "w1C"):
                w[k] = wpool.tile_from(wd[k][:], name=k)
            xs[1] = xpool.tile([3, B], F32R, tag="xs", name="xs")
            nc.sync.dma_start(out=xs[1][:], in_=obs_d[1])
            for k, v in wd.items():
                if k not in ("ones", "ones_bf") and k not in w:
                    w[k] = wpool.tile_from(v[:], name=k)

            obsA = rpool.tile([128, B], F32R, tag="obsA")   # ring slots 0,1
            obsC = rpool.tile([65, B], F32R, tag="obsC")    # slot 2 + ones row
            c1A = rpool.tile([128, B], F32R, tag="c1A")
            c1C = rpool.tile([65, B], F32R, tag="c1C")
            c2r = rpool.tile([97, B], F32R, tag="c2r")      # 3 bands + ones row
            S_all = rpool.tile([128, NSLOT, B], BF16, tag="S_all")
            # MX split even/odd so each tile has a ones row (row 64) that
            # carries c_d / b_hp through the decB66 matmul.
            MXe = rpool.tile([65, NSLOT, NS], BF16, tag="MXe")
            MXo = rpool.tile([65, NSLOT, NS], BF16, tag="MXo")
            relout = rpool.tile([24, B], F32, tag="relout")
            st1 = [rpool.tile([64, NS // 4, 4], BF16, tag=f"st1_{i}",
                              name=f"st1_{i}") for i in range(2)]
            st2 = [rpool.tile([64, NS // 4, 2], BF16, tag=f"st2_{i}",
                              name=f"st2_{i}") for i in range(2)]

            # memset doesn't support float32r on HW ISA — DMA the ones rows
            nc.sync.dma_start(out=obsC[64:65, :], in_=wd["ones"][:])
            nc.sync.dma_start(out=c1C[64:65, :], in_=wd["ones"][:])
            nc.sync.dma_start(out=c2r[96:97, :], in_=wd["ones"][:])
            nc.sync.dma_start(
                out=MXe[64:65, :, :],
                in_=wd["ones_bf"][:, :NSLOT * NS].rearrange(
                    "o (s n) -> o s n", s=NSLOT))
            nc.sync.dma_start(
                out=MXo[64:65, :, :],
                in_=wd["ones_bf"][:, NSLOT * NS:].rearrange(
                    "o (s n) -> o s n", s=NSLOT))

            def eng(sel):
                return {'v': nc.vector, 'a': nc.scalar, 'p': nc.gpsimd}[sel]

            def copy_op(sel, out, in_):
                if sel == 'a':
                    nc.scalar.activation(
                        out, in_, mybir.ActivationFunctionType.Identity)
                else:
                    eng(sel).tensor_copy(out=out, in_=in_)

            def relu_op(sel, out, in_):
                if sel == 'a':
                    nc.scalar.activation(
                        out, in_, mybir.ActivationFunctionType.Relu)
                else:
                    eng(sel).tensor_scalar_max(out, in_, 0.0)

            def obs_slot(j, ci):
                sl = slice(ci * CH, (ci + 1) * CH)
                if j == 0:
                    return obsA[0:64, sl]
                if j == 1:
                    return obsA[64:128, sl]
                return obsC[0:64, sl]

            def c1_slot(j, ci):
                sl = slice(ci * CH, (ci + 1) * CH)
                if j == 0:
                    return c1A[0:64, sl]
                if j == 1:
                    return c1A[64:128, sl]
                return c1C[0:64, sl]

            def mx_tile(s):
                return MXe if s % 2 == 0 else MXo

            sbRelMx = rpool.tile([24, NS], F32, tag="sbRelMx")

            def stage1(g, ci):
                sl = slice(ci * CH, (ci + 1) * CH)
                ssl = slice(ci * (CH // SCENE), (ci + 1) * (CH // SCENE))
                if g < T:
                    if KNOBS['borrow'] and g < 4:
                        pool_rr = [pdec, pc2, pc3, psBp][ci % 4]
                        tag_rr = ["psdec", "psc2", "psc3", "psB"][ci % 4]
                    elif KNOBS['borrow'] and g < 6:
                        pool_rr = [pdec, pc3, psBp][ci % 3]
                        tag_rr = ["psdec", "psc3", "psB"][ci % 3]
                    elif KNOBS['borrow']:
                        pool_rr = [pdec, psBp][ci % 2]
                        tag_rr = ["psdec", "psB"][ci % 2]
                    else:
                        pool_rr, tag_rr = pdec, "psdec"
                    ps = pool_rr.tile([64, CH], F32, tag=tag_rr, name="psE")
                    nc.tensor.matmul(ps[:], w["wse_t"][:],
                                     xs[g % 2][:, sl], start=True, stop=True)
                    copy_op(KNOBS['embed'][ci % len(KNOBS['embed'])], obs_slot(g % 3, ci), ps[:])
                else:
                    s = g - T
                    band = (s % 2) * 64
                    slot = s // 2
                    ps = pdec.tile([64, CH], F32, tag="psdec", name="psA")
                    nc.tensor.matmul(
                        ps[:], w["decA"][band:band + 64, :],
                        S_all[band:band + 64, slot, sl],
                        start=True, stop=False)
                    mxb = (mx_tile(s)[:, slot, ssl].unsqueeze(2)
                           .broadcast_to((65, CH // SCENE, SCENE)))
                    nc.tensor.matmul(ps[:], w["decB65"][:], mxb,
                                     start=False, stop=True)
                    da = KNOBS['dec_add']
                    copy_op(da[ci % len(da)], obs_slot(g % 3, ci), ps[:])

            def conv1(g, ci):
                sl = slice(ci * CH, (ci + 1) * CH)
                p = g - 2
                r = p % 3
                ps1 = pc1.tile([64, CH], F32, tag="psc1")
                # read the stale ring tile first: the tile holding this
                # iteration's fresh column gates on the stage-1 add
                mmA = (w["w1A"][:, r * 64:(r + 1) * 64], obsA[:, sl])
                mmC = (w["w1C"][:, r * 64:(r + 1) * 64], obsC[:, sl])
                first, second = (mmC, mmA) if g % 3 != 2 else (mmA, mmC)
                nc.tensor.matmul(ps1[:], first[0], first[1],
                                 start=True, stop=False)
                nc.tensor.matmul(ps1[:], second[0], second[1],
                                 start=False, stop=True)
                r = KNOBS['r1']
                relu_op(r[ci % len(r)], c1_slot(p % 3, ci), ps1[:])

            ps2_pair = [None]

            def conv2(g, ci):
                sl = slice(ci * CH, (ci + 1) * CH)
                q = g - 4
                r = q % 3
                band2 = (q % 3) * 32
                if ci % 2 == 0:
                    ps2_pair[0] = pc2.tile([32, 2 * CH], F32, tag="psc2",
                                           name="ps2")
                half = ps2_pair[0][:, (ci % 2) * CH:(ci % 2) * CH + CH]
                nc.tensor.matmul(half,
                                 w["w2A"][:, r * 32:(r + 1) * 32],
                                 c1A[:, sl], start=True, stop=False)
                nc.tensor.matmul(half,
                                 w["w2C"][:, r * 32:(r + 1) * 32],
                                 c1C[:, sl], start=False, stop=True)
                if ci % 2 == 1:
                    slp = slice((ci - 1) * CH, (ci + 1) * CH)
                    r = KNOBS['r2']
                    relu_op(r[(ci // 2) % len(r)],
                            c2r[band2:band2 + 32, slp], ps2_pair[0][:])

            ps3_last = [None]

            def conv3_mm(g, ci):
                sl = slice(ci * CH, (ci + 1) * CH)
                u = g - 6
                r = u % 3
                ps3_last[0] = pc3.tile([32, CH], F32, tag="psc3", name="ps3")
                ps3 = ps3_last[0]
                nc.tensor.matmul(ps3[:],
                                 w["w3A"][:, r * 32:(r + 1) * 32],
                                 c2r[:, sl], start=True, stop=True)
                if u <= SEQ - 1:
                    b0 = (u % 2) * 64
                    r = KNOBS['r3']
                    relu_op(r[ci % len(r)], S_all[b0:b0 + 32, u // 2, sl],
                            ps3[:])

            def conv3_post(g, ci):
                # pair-granular (runs at odd ci over chunks ci-1, ci): wide
                # hi copy on DVE, wide max-tree on the knob engine —
                # halves per-op overhead and keeps Pool SBUF-only
                if ci % 2 == 0:
                    return
                slp = slice((ci - 1) * CH, (ci + 1) * CH)
                sslp = slice((ci - 1) * (CH // SCENE),
                             (ci + 1) * (CH // SCENE))
                u = g - 6
                if 1 <= u:
                    k = u - 1
                    b1_ = (k % 2) * 64 + 32
                    if u <= SEQ - 1:
                        copy_op(KNOBS['hi'],
                                S_all[b1_:b1_ + 32, k // 2, slp],
                                S_all[(u % 2) * 64:(u % 2) * 64 + 32,
                                      u // 2, slp])
                    else:
                        for cj in (ci - 1, ci):
                            relu_op('v',
                                    S_all[b1_:b1_ + 32, k // 2,
                                          cj * CH:(cj + 1) * CH],
                                    ps3_cache[cj][:])
                if g < 7:
                    return
                s2 = g - 7
                band = (s2 % 2) * 64
                sv = (S_all[band:band + 64, s2 // 2, slp]
                      .rearrange("p (s e) -> p s e", e=SCENE))
                if KNOBS['seg'] == 'tree':
                    se = eng(KNOBS['seg_eng'])
                    t1, t2 = st1[(ci // 2) % 2], st2[(ci // 2) % 2]
                    se.tensor_max(t1[:], sv[:, :, 0:4], sv[:, :, 4:8])
                    se.tensor_max(t2[:], t1[:, :, 0:2], t1[:, :, 2:4])
                    se.tensor_max(
                        mx_tile(s2)[0:64, s2 // 2, sslp],
                        t2[:, :, 0], t2[:, :, 1])
                else:
                    nc.vector.reduce_max(
                        out=mx_tile(s2)[0:64, s2 // 2, sslp],
                        in_=sv, axis=mybir.AxisListType.X)

            # software-pipelined emission: stage1 of chunk i is emitted
            # alongside conv1 of i-1, conv2 of i-2, conv3 of i-3 so every
            # PE->vector->PE hop has several matmuls of slack
            ps3_cache = [None] * NCHUNK
            for g in range(T + SEQ - 1):           # g = 0..18
                for si in range(NCHUNK + 3):
                    if si < NCHUNK:
                        stage1(g, si)
                    if g >= 2 and 1 <= si <= NCHUNK:
                        conv1(g, si - 1)
                    if g >= 4 and 2 <= si <= NCHUNK + 1:
                        conv2(g, si - 2)
                    if g >= 6 and 3 <= si <= NCHUNK + 2:
                        conv3_mm(g, si - 3)
                        ps3_cache[si - 3] = ps3_last[0]
                        conv3_post(g, si - 3)
                if g + 2 < T:
                    xs[g % 2] = xpool.tile([3, B], F32R, tag="xs", name="xs")
                    nc.sync.dma_start(out=xs[g % 2][:], in_=obs_d[g + 2])
                # ---- shared per-step scene contribution for step s=g-7 ----

            # ---- endgame: rels for all 12 steps ----
            # shared scene contribution: psRelMx (24, NS) accumulated over
            # all 12 half-slots (b_hp rides relBe slot 0's ones row)
            psRelMx = psBp.tile([24, CH], F32, tag="psB")
            for slot in range(NSLOT):
                nc.tensor.matmul(psRelMx[:],
                                 w["relBe"][:, slot * 24:(slot + 1) * 24],
                                 MXe[:, slot, :],
                                 start=(slot == 0), stop=False)
                nc.tensor.matmul(psRelMx[:],
                                 w["relBo"][:, slot * 24:(slot + 1) * 24],
                                 MXo[:, slot, :],
                                 start=False, stop=(slot == NSLOT - 1))
            nc.vector.tensor_copy(out=sbRelMx[:], in_=psRelMx[:])
            for ci in range(NCHUNK):
                sl = slice(ci * CH, (ci + 1) * CH)
                ssl = slice(ci * (CH // SCENE), (ci + 1) * (CH // SCENE))
                pool_e = [pdec, pc1, pc3, pc2][ci % 4]
                tag_e = ["psdec", "psc1", "psc3", "psc2"][ci % 4]
                ps = pool_e.tile([24, CH], F32, tag=tag_e, name="psR")
                for slot in range(NSLOT):
                    nc.tensor.matmul(
                        ps[:], w["relA"][:, slot * 24:(slot + 1) * 24],
                        S_all[:, slot, sl],
                        start=(slot == 0), stop=(slot == NSLOT - 1))
                rmx = (sbRelMx[:, ssl].unsqueeze(2)
                       .broadcast_to((24, CH // SCENE, SCENE)))
                cmb = KNOBS['combine']
                eng(cmb[ci % len(cmb)]).tensor_add(relout[:, sl], ps[:], rmx)
                nc.sync.dma_start(out=rels_d[:, sl], in_=relout[:, sl])

    nc.compile()   # bacc passes: split multi-waits into EventSemaphores etc.
    return nc


def _numpy_fallback(obs_traj, W_se, b_se, v1, g1, b1, v2, g2, b2, v3, g3, b3,
                    W_hp, b_hp, seq_start_end, seq_len):
    """Exact numpy implementation for inputs the device kernel wasn't built
    for (non-uniform segments / different seq_len)."""
    batch = obs_traj.shape[1]
    nseg = seq_start_end.shape[0]
    seg = np.searchsorted(seq_start_end[:, 0], np.arange(batch),
                          side="right") - 1

    def wn(v, g):
        n = np.sqrt((v * v).sum(axis=(1, 2)))
        return v * (g / n)[:, None, None]

    w1, w2, w3 = wn(v1, g1), wn(v2, g2), wn(v3, g3)

    def conv(x, w, b):
        O = w.shape[0]
        Tn = x.shape[2]
        out = np.zeros((x.shape[0], O, Tn - 2), np.float32)
        for t in range(Tn - 2):
            for k in range(3):
                out[:, :, t] += x[:, :, t + k] @ w[:, :, k].T
        return np.maximum(out + b[None, :, None], 0)

    emb = obs_traj @ W_se.T + b_se
    obs_emb = np.transpose(emb, (1, 2, 0)).copy()
    rels = []
    for _ in range(int(seq_len)):
        c3 = conv(conv(conv(obs_emb, w1, b1), w2, b2), w3, b3)
        s = c3.reshape(batch, 64)
        mx = np.full((nseg, 64), -np.inf, np.float32)
        np.maximum.at(mx, seg, s)
        st = np.concatenate([s, mx[seg]], axis=1)
        rel = st @ W_hp.T + b_hp
        dec = rel @ W_se.T + b_se
        obs_emb = np.concatenate([obs_emb[:, :, 1:], dec[:, :, None]], axis=2)
        rels.append(rel)
    return np.stack(rels).astype(np.float32)


def kernel(obs_traj, last_pos, last_pos_rel, W_se, b_se, v1, g1, b1,
           v2, g2, b2, v3, g3, b3, W_hp, b_hp, seq_start_end, seq_len):
    obs_traj = np.asarray(obs_traj, np.float32)
    seq_start_end = np.asarray(seq_start_end)
    args = [np.asarray(a, np.float32) for a in
            (W_se, b_se, v1, g1, b1, v2, g2, b2, v3, g3, b3, W_hp, b_hp)]

    starts = np.arange(BATCH // SCENE, dtype=np.int64) * SCENE
    uniform = (obs_traj.shape == (T, BATCH, 2)
               and int(seq_len) == SEQ
               and seq_start_end.shape == (BATCH // SCENE, 2)
               and np.array_equal(seq_start_end[:, 0], starts)
               and np.array_equal(seq_start_end[:, 1], starts + SCENE))
    if not uniform:
        return _numpy_fallback(obs_traj, *args, seq_start_end, seq_len)

    if "nc" not in _cache:
        _cache["nc"] = _build_module()
    nc = _cache["nc"]

    wdev = _host_weights(*args)
    obs_t = np.concatenate([obs_traj.transpose(0, 2, 1),
                            np.ones((T, 1, BATCH), np.float32)],
                           axis=1)  # (8, 3, 32768) with ones plane

    in_maps = []
    for core in range(NCORES):
        m = dict(wdev)
        m["obs"] = np.ascontiguousarray(obs_t[:, :, core * B:(core + 1) * B])
        in_maps.append(m)

    res = run_bass_kernel_spmd(nc, in_maps, core_ids=list(range(NCORES)))

    out = np.empty((SEQ, BATCH, 2), np.float32)
    for core in range(NCORES):
        arr = res.results[core]["rels"]          # (24, B)
        for c in range(2):
            out[:, core * B:(core + 1) * B, c] = arr[c::2]
    return out


# revision 44
# speedup vs baseline: 1.1450x; 1.0038x over previous
"""Trainium2 Bass kernel for nn_Encoder_66872640799015 (segment_reduce).

Recurrent conv encoder over 32768 pedestrians (4096 scenes x 8), 12 steps.
Sharding: data-parallel over scenes — 8 cores x 4096 pedestrians (512 whole
scenes per core), weights replicated.

Algorithmic structure (v2):
- Rolling ring buffers: each scan step shifts the conv window by one column,
  so only ONE new conv position per layer per step is computed.  Weights are
  pre-permuted per ring rotation; conv biases ride as ones-row lhsT rows so
  every relu is a bias-free single op placeable on any engine.
- Software-pipelined emission: per iteration, stage1 of chunk i is emitted
  alongside conv1 of i-1, conv2 of i-2 and conv3 of i-3, so each
  PE->vector->PE dependency hop has several matmuls of slack and the
  in-order engine queues never block.
- dec's scene-max term is a broadcast-rhs matmul over 65-row MXe/MXo tiles
  (ones row carries c_d), accumulated into the same PSUM as the A-part, so
  the ring write is a single copy; the endgame's MX term is contracted once
  over the 512 distinct scenes and broadcast-added from SBUF.
- Segment-max is a 3-op pairwise bf16 max tree on DVE (2x packed mode) over
  chunk pairs instead of a full-rate tensor_reduce.
- TRN2 legality: GpSimd(Pool) touches no PSUM and runs no TensorTensor;
  no DVE broadcast operand reads PSUM (scene terms are staged via SBUF).
"""

import sys

sys.path.insert(0, "/opt/trn_rl_repo")

import numpy as np
import ml_dtypes

import concourse.bass as bass
import concourse.bacc as bacc
import concourse.tile as tile
from concourse import mybir
from concourse.bass_utils import run_bass_kernel_spmd

NCORES = 8
BATCH = 32768
B = BATCH // NCORES        # pedestrians per core
T = 8                      # obs_len
SEQ = 12                   # seq_len
SCENE = 8                  # pedestrians per scene
NS = B // SCENE            # scenes per core (512)
CH = 512                   # free-dim chunk (one PSUM bank of fp32)
NCHUNK = B // CH
NSLOT = SEQ // 2           # S_all free slots (2 steps per slot)

F32 = mybir.dt.float32
F32R = mybir.dt.float32r
BF16 = mybir.dt.bfloat16

_cache = {}

# engine-assignment knobs: which engine runs each elementwise op
# 'v' = DVE (vector), 'a' = ACT (scalar), 'p' = Pool (gpsimd)
KNOBS = dict(
    embed='v',       # engine rotation for embed-phase psum->ring copies
    dec_add='v',     # engine rotation for the dec psum->ring copies
    combine='v',
    r1='a', r2='a', r3='av',
    hi='v',
    seg='tree',      # 'reduce' | 'tree'
    seg_eng='p',     # engine for the max tree (SBUF-only op)
    psum=(2, 1, 2, 1, 1),   # (pdec, psB, pc1, pc2-wide, pc3)
    borrow=True,     # embed borrows idle conv psum pools during fill
    special7=False,   # split max-tree for chunk 7 (psB chain shortening)
)


def _perm(r):
    """S-feature row (32*t + ch) -> reference feature index (2*ch + t)."""
    t, ch = r // 32, r % 32
    return 2 * ch + t


def _host_weights(W_se, b_se, v1, g1, b1, v2, g2, b2, v3, g3, b3, W_hp, b_hp):
    """Derive all device weight tensors (pre-permuted / rotation variants)."""
    f32 = np.float32

    def wn(v, g):
        n = np.sqrt((v * v).sum(axis=(1, 2)))
        return (v * (g / n)[:, None, None]).astype(f32)

    w1 = wn(v1, g1)   # (64, 64, 3)
    w2 = wn(v2, g2)   # (32, 64, 3)
    w3 = wn(v3, g3)   # (32, 32, 3)

    # conv lhsT rotation variants.  Ring slot j holds tap k = (j - r) mod 3
    # where r is the rotation (= conv position mod 3).
    def conv_variants(w, nin, nout, nslots):
        # returns (nslots*nin, 3, nout): [slot-block rows, rotation, out]
        out = np.zeros((nslots * nin, 3, nout), f32)
        for r in range(3):
            for j in range(nslots):
                k = (j - r) % 3
                # lhsT rows = input channels of slot j, cols = out channel
                out[j * nin:(j + 1) * nin, r, :] = w[:, :, k].T
        return out

    w1A = conv_variants(w1, 64, 64, 2)            # (128, 3, 64) slots 0,1
    w1C = conv_variants(w1, 64, 64, 3)[128:]      # (64, 3, 64)  slot 2
    w2A = conv_variants(w2, 64, 32, 2)            # (128, 3, 32)
    w2C = conv_variants(w2, 64, 32, 3)[128:]      # (64, 3, 32)
    w3A = conv_variants(w3, 32, 32, 3)            # (96, 3, 32)
    # bias rows: ring tiles carry a constant ones-row as an extra partition,
    # so the conv bias rides in the matmul (lhsT bottom row) and the relus
    # become bias-free single ops placeable on any engine.
    w1C = np.concatenate([w1C, np.tile(b1.reshape(1, 1, 64), (1, 3, 1))], 0)
    w2C = np.concatenate([w2C, np.tile(b2.reshape(1, 1, 32), (1, 3, 1))], 0)
    w3A = np.concatenate([w3A, np.tile(b3.reshape(1, 1, 32), (1, 3, 1))], 0)

    perm = np.array([_perm(r) for r in range(64)])

    # dec = A_mat @ s + Bm_mat @ mx[seg] + c_d   (feedback column, 64-dim)
    # rel = W_hpa @ s + W_hpb @ mx[seg] + b_hp   (2-dim, rides as 2 extra
    # output columns on the same matmuls)
    W_hpa, W_hpb = W_hp[:, :64], W_hp[:, 64:]
    A_mat = (W_se @ W_hpa).astype(f32)    # (64 emb, 64 feat)
    Bm_mat = (W_se @ W_hpb).astype(f32)
    c_d = (W_se @ b_hp + b_se).astype(f32)

    # decA: (128, 64) doubled vertically so lhsT can be sliced at base
    # partition 0 or 64 to match the S band of even/odd steps.
    decA = np.vstack([A_mat[:, perm].T] * 2).copy()        # (128, 64)

    # decB65: (65, 64): rows 0:64 contract the 64 MX rows, row 64 is the
    # ones row -> carries c_d.
    decB65 = np.zeros((65, 64), f32)
    decB65[:64, :] = Bm_mat[:, perm].T
    decB65[64, :] = c_d

    # rel endgame: out partition p = 2*k + c (k=step, c=coord).
    # relA lhsT per slot: (128 rows = [band0: step 2*slot, band1: step
    # 2*slot+1] feature rows, 24 cols).  relBe/relBo contract the 65-row
    # MXe/MXo tiles per slot; b_hp rides the ones row of relBe slot 0.
    relA = np.zeros((128, NSLOT, 24), f32)
    relBe = np.zeros((65, NSLOT, 24), f32)
    relBo = np.zeros((65, NSLOT, 24), f32)
    for slot in range(NSLOT):
        for band in range(2):
            k = 2 * slot + band
            rows = slice(band * 64, band * 64 + 64)
            dst = relBe if band == 0 else relBo
            for c in range(2):
                relA[rows, slot, 2 * k + c] = W_hpa[c, perm]
                dst[:64, slot, 2 * k + c] = W_hpb[c, perm]
    relBe[64, 0, :] = np.tile(b_hp, SEQ)

    bf = ml_dtypes.bfloat16
    return {
        "wse_t": np.concatenate(
            [np.ascontiguousarray(W_se.T, f32), b_se.reshape(1, 64)], 0),
        "w1A": w1A.reshape(128, 3 * 64),
        "w1C": w1C.reshape(65, 3 * 64),
        "w2A": w2A.reshape(128, 3 * 32),
        "w2C": w2C.reshape(65, 3 * 32),
        "w3A": w3A.reshape(97, 3 * 32),
        "decA": decA.astype(bf),
        "decB65": decB65.astype(bf),
        "relA": relA.reshape(128, NSLOT * 24).astype(bf),
        "relBe": relBe.reshape(65, NSLOT * 24).astype(bf),
        "relBo": relBo.reshape(65, NSLOT * 24).astype(bf),
        "ones": np.ones((1, B), np.float32),
        "ones_bf": np.ones((1, 2 * NSLOT * NS), bf),
    }


def _build_module():
    """Build the SPMD Bass module (input-independent, cached)."""
    nc = bacc.Bacc()

    obs_d = nc.dram_tensor("obs", [T, 3, B], F32R, kind="ExternalInput")
    wd = {}
    for name, p, f, dt in [
        ("wse_t", 3, 64, F32R), ("w1A", 128, 192, F32R), ("w1C", 65, 192, F32R),
        ("w2A", 128, 96, F32R), ("w2C", 65, 96, F32R), ("w3A", 97, 96, F32R),
        ("decA", 128, 64, BF16), ("decB65", 65, 64, BF16),
        ("relA", 128, NSLOT * 24, BF16), ("relBe", 65, NSLOT * 24, BF16),
        ("relBo", 65, NSLOT * 24, BF16),
        ("ones", 1, B, F32R), ("ones_bf", 1, 2 * NSLOT * NS, BF16),
    ]:
        wd[name] = nc.dram_tensor(name, [p, f], dt, kind="ExternalInput")
    rels_d = nc.dram_tensor("rels", [24, B], F32, kind="ExternalOutput")

    with tile.TileContext(nc) as tc:
        with (
            tc.tile_pool(name="weights", bufs=1) as wpool,
            tc.tile_pool(name="rings", bufs=1) as rpool,
            tc.tile_pool(name="stage", bufs=2) as xpool,
            tc.tile_pool(name="pdec", bufs=KNOBS['psum'][0], space="PSUM") as pdec,
            tc.tile_pool(name="psB", bufs=KNOBS['psum'][1], space="PSUM") as psBp,
            tc.tile_pool(name="pc1", bufs=KNOBS['psum'][2], space="PSUM") as pc1,
            tc.tile_pool(name="pc2", bufs=KNOBS['psum'][3], space="PSUM") as pc2,
            tc.tile_pool(name="pc3", bufs=KNOBS['psum'][4], space="PSUM") as pc3,
        ):
            # DMA issue order = HWDGE service order: load the tensors the
            # pipeline needs first (obs column 0, embed + conv1 weights)
            xs = [None, None]
            xs[0] = xpool.tile([3, B], F32R, tag="xs", name="xs")
            nc.sync.dma_start(out=xs[0][:], in_=obs_d[0])
            w = {}
            for k in ("wse_t", "w1A", "w1C"):
                w[k] = wpool.tile_from(wd[k][:], name=k)
            xs[1] = xpool.tile([3, B], F32R, tag="xs", name="xs")
            nc.sync.dma_start(out=xs[1][:], in_=obs_d[1])
            for k, v in wd.items():
                if k not in ("ones", "ones_bf") and k not in w:
                    w[k] = wpool.tile_from(v[:], name=k)

            obsA = rpool.tile([128, B], F32R, tag="obsA")   # ring slots 0,1
            obsC = rpool.tile([65, B], F32R, tag="obsC")    # slot 2 + ones row
            c1A = rpool.tile([128, B], F32R, tag="c1A")
            c1C = rpool.tile([65, B], F32R, tag="c1C")
            c2r = rpool.tile([97, B], F32R, tag="c2r")      # 3 bands + ones row
            S_all = rpool.tile([128, NSLOT, B], BF16, tag="S_all")
            # MX split even/odd so each tile has a ones row (row 64) that
            # carries c_d / b_hp through the decB66 matmul.
            MXe = rpool.tile([65, NSLOT, NS], BF16, tag="MXe")
            MXo = rpool.tile([65, NSLOT, NS], BF16, tag="MXo")
            relout = rpool.tile([24, B], F32, tag="relout")
            st1 = [rpool.tile([64, NS // 4, 4], BF16, tag=f"st1_{i}",
                              name=f"st1_{i}") for i in range(2)]
            st2 = [rpool.tile([64, NS // 4, 2], BF16, tag=f"st2_{i}",
                              name=f"st2_{i}") for i in range(2)]

            # memset doesn't support float32r on HW ISA — DMA the ones rows
            nc.sync.dma_start(out=obsC[64:65, :], in_=wd["ones"][:])
            nc.sync.dma_start(out=c1C[64:65, :], in_=wd["ones"][:])
            nc.sync.dma_start(out=c2r[96:97, :], in_=wd["ones"][:])
            nc.sync.dma_start(
                out=MXe[64:65, :, :],
                in_=wd["ones_bf"][:, :NSLOT * NS].rearrange(
                    "o (s n) -> o s n", s=NSLOT))
            nc.sync.dma_start(
                out=MXo[64:65, :, :],
                in_=wd["ones_bf"][:, NSLOT * NS:].rearrange(
                    "o (s n) -> o s n", s=NSLOT))

            def eng(sel):
                return {'v': nc.vector, 'a': nc.scalar, 'p': nc.gpsimd}[sel]

            def copy_op(sel, out, in_):
                if sel == 'a':
                    nc.scalar.activation(
                        out, in_, mybir.ActivationFunctionType.Identity)
                else:
                    eng(sel).tensor_copy(out=out, in_=in_)

            def relu_op(sel, out, in_):
                if sel == 'a':
                    nc.scalar.activation(
                        out, in_, mybir.ActivationFunctionType.Relu)
                else:
                    eng(sel).tensor_scalar_max(out, in_, 0.0)

            def obs_slot(j, ci):
                sl = slice(ci * CH, (ci + 1) * CH)
                if j == 0:
                    return obsA[0:64, sl]
                if j == 1:
                    return obsA[64:128, sl]
                return obsC[0:64, sl]

            def c1_slot(j, ci):
                sl = slice(ci * CH, (ci + 1) * CH)
                if j == 0:
                    return c1A[0:64, sl]
                if j == 1:
                    return c1A[64:128, sl]
                return c1C[0:64, sl]

            def mx_tile(s):
                return MXe if s % 2 == 0 else MXo

            sbRelMx = rpool.tile([24, NS], F32, tag="sbRelMx")
            psRelMx = [None]

            def stage1(g, ci):
                sl = slice(ci * CH, (ci + 1) * CH)
                ssl = slice(ci * (CH // SCENE), (ci + 1) * (CH // SCENE))
                if g < T:
                    if KNOBS['borrow'] and g < 4:
                        pool_rr = [pdec, pc2, pc3, psBp][ci % 4]
                        tag_rr = ["psdec", "psc2", "psc3", "psB"][ci % 4]
                    elif KNOBS['borrow'] and g < 6:
                        pool_rr = [pdec, pc3, psBp][ci % 3]
                        tag_rr = ["psdec", "psc3", "psB"][ci % 3]
                    elif KNOBS['borrow']:
                        pool_rr = [pdec, psBp][ci % 2]
                        tag_rr = ["psdec", "psB"][ci % 2]
                    else:
                        pool_rr, tag_rr = pdec, "psdec"
                    ps = pool_rr.tile([64, CH], F32, tag=tag_rr, name="psE")
                    nc.tensor.matmul(ps[:], w["wse_t"][:],
                                     xs[g % 2][:, sl], start=True, stop=True)
                    copy_op(KNOBS['embed'][ci % len(KNOBS['embed'])], obs_slot(g % 3, ci), ps[:])
                else:
                    s = g - T
                    band = (s % 2) * 64
                    slot = s // 2
                    ps = pdec.tile([64, CH], F32, tag="psdec", name="psA")
                    nc.tensor.matmul(
                        ps[:], w["decA"][band:band + 64, :],
                        S_all[band:band + 64, slot, sl],
                        start=True, stop=False)
                    mxb = (mx_tile(s)[:, slot, ssl].unsqueeze(2)
                           .broadcast_to((65, CH // SCENE, SCENE)))
                    nc.tensor.matmul(ps[:], w["decB65"][:], mxb,
                                     start=False, stop=True)
                    da = KNOBS['dec_add']
                    copy_op(da[ci % len(da)], obs_slot(g % 3, ci), ps[:])

            def conv1(g, ci):
                sl = slice(ci * CH, (ci + 1) * CH)
                p = g - 2
                r = p % 3
                ps1 = pc1.tile([64, CH], F32, tag="psc1")
                # read the stale ring tile first: the tile holding this
                # iteration's fresh column gates on the stage-1 add
                mmA = (w["w1A"][:, r * 64:(r + 1) * 64], obsA[:, sl])
                mmC = (w["w1C"][:, r * 64:(r + 1) * 64], obsC[:, sl])
                first, second = (mmC, mmA) if g % 3 != 2 else (mmA, mmC)
                nc.tensor.matmul(ps1[:], first[0], first[1],
                                 start=True, stop=False)
                nc.tensor.matmul(ps1[:], second[0], second[1],
                                 start=False, stop=True)
                r = KNOBS['r1']
                relu_op(r[ci % len(r)], c1_slot(p % 3, ci), ps1[:])

            ps2_pair = [None]

            def conv2(g, ci):
                sl = slice(ci * CH, (ci + 1) * CH)
                q = g - 4
                r = q % 3
                band2 = (q % 3) * 32
                if ci % 2 == 0:
                    ps2_pair[0] = pc2.tile([32, 2 * CH], F32, tag="psc2",
                                           name="ps2")
                half = ps2_pair[0][:, (ci % 2) * CH:(ci % 2) * CH + CH]
                nc.tensor.matmul(half,
                                 w["w2A"][:, r * 32:(r + 1) * 32],
                                 c1A[:, sl], start=True, stop=False)
                nc.tensor.matmul(half,
                                 w["w2C"][:, r * 32:(r + 1) * 32],
                                 c1C[:, sl], start=False, stop=True)
                if ci % 2 == 1:
                    slp = slice((ci - 1) * CH, (ci + 1) * CH)
                    r = KNOBS['r2']
                    relu_op(r[(ci // 2) % len(r)],
                            c2r[band2:band2 + 32, slp], ps2_pair[0][:])

            ps3_last = [None]

            def conv3_mm(g, ci):
                sl = slice(ci * CH, (ci + 1) * CH)
                u = g - 6
                r = u % 3
                ps3_last[0] = pc3.tile([32, CH], F32, tag="psc3", name="ps3")
                ps3 = ps3_last[0]
                nc.tensor.matmul(ps3[:],
                                 w["w3A"][:, r * 32:(r + 1) * 32],
                                 c2r[:, sl], start=True, stop=True)
                if u <= SEQ - 1:
                    b0 = (u % 2) * 64
                    r = KNOBS['r3']
                    relu_op(r[ci % len(r)], S_all[b0:b0 + 32, u // 2, sl],
                            ps3[:])

            def conv3_post(g, ci):
                # pair-granular (runs at odd ci over chunks ci-1, ci): wide
                # hi copy on DVE, wide max-tree on the knob engine —
                # halves per-op overhead and keeps Pool SBUF-only
                if ci % 2 == 0:
                    return
                slp = slice((ci - 1) * CH, (ci + 1) * CH)
                sslp = slice((ci - 1) * (CH // SCENE),
                             (ci + 1) * (CH // SCENE))
                u = g - 6
                if 1 <= u:
                    k = u - 1
                    b1_ = (k % 2) * 64 + 32
                    if u <= SEQ - 1:
                        copy_op(KNOBS['hi'],
                                S_all[b1_:b1_ + 32, k // 2, slp],
                                S_all[(u % 2) * 64:(u % 2) * 64 + 32,
                                      u // 2, slp])
                    else:
                        for cj in (ci - 1, ci):
                            relu_op('v',
                                    S_all[b1_:b1_ + 32, k // 2,
                                          cj * CH:(cj + 1) * CH],
                                    ps3_cache[cj][:])
                if g < 7:
                    return
                s2 = g - 7
                band = (s2 % 2) * 64
                sv = (S_all[band:band + 64, s2 // 2, slp]
                      .rearrange("p (s e) -> p s e", e=SCENE))
                if KNOBS['seg'] == 'tree':
                    se = eng(KNOBS['seg_eng'])
                    t1, t2 = st1[(ci // 2) % 2], st2[(ci // 2) % 2]
                    se.tensor_max(t1[:], sv[:, :, 0:4], sv[:, :, 4:8])
                    se.tensor_max(t2[:], t1[:, :, 0:2], t1[:, :, 2:4])
                    se.tensor_max(
                        mx_tile(s2)[0:64, s2 // 2, sslp],
                        t2[:, :, 0], t2[:, :, 1])
                else:
                    nc.vector.reduce_max(
                        out=mx_tile(s2)[0:64, s2 // 2, sslp],
                        in_=sv, axis=mybir.AxisListType.X)

            # software-pipelined emission: stage1 of chunk i is emitted
            # alongside conv1 of i-1, conv2 of i-2, conv3 of i-3 so every
            # PE->vector->PE hop has several matmuls of slack
            ps3_cache = [None] * NCHUNK
            for g in range(T + SEQ - 1):           # g = 0..18
                for si in range(NCHUNK + 3):
                    if g == T + SEQ - 2 and si < NCHUNK + 3:
                        mm = [(w["relBe"], MXe, s_, s_ == 0, False)
                              for s_ in range(NSLOT)] + \
                             [(w["relBo"], MXo, s_, False, False)
                              for s_ in range(NSLOT - 1)]
                        if si < len(mm):
                            wt, mxt, s_, st, sp = mm[si]
                            if si == 0:
                                psRelMx[0] = psBp.tile(
                                    [24, CH], F32, tag="psB", name="psRelMx")
                            nc.tensor.matmul(
                                psRelMx[0][:], wt[:, s_ * 24:(s_ + 1) * 24],
                                mxt[:, s_, :], start=st, stop=sp)
                    if si < NCHUNK:
                        stage1(g, si)
                    if g >= 2 and 1 <= si <= NCHUNK:
                        conv1(g, si - 1)
                    if g >= 4 and 2 <= si <= NCHUNK + 1:
                        conv2(g, si - 2)
                    if g >= 6 and 3 <= si <= NCHUNK + 2:
                        conv3_mm(g, si - 3)
                        ps3_cache[si - 3] = ps3_last[0]
                        conv3_post(g, si - 3)
                if g + 2 < T:
                    xs[g % 2] = xpool.tile([3, B], F32R, tag="xs", name="xs")
                    nc.sync.dma_start(out=xs[g % 2][:], in_=obs_d[g + 2])
                # ---- shared per-step scene contribution for step s=g-7 ----

            # ---- endgame: rels for all 12 steps ----
            # psRelMx was accumulated during g=18 (see loop); finish with
            # the step-11 contribution and stage to SBUF
            nc.tensor.matmul(psRelMx[0][:],
                             w["relBo"][:, 5 * 24:6 * 24],
                             MXo[:, 5, :], start=False, stop=True)
            nc.vector.tensor_copy(out=sbRelMx[:], in_=psRelMx[0][:])
            for ci in range(NCHUNK):
                sl = slice(ci * CH, (ci + 1) * CH)
                ssl = slice(ci * (CH // SCENE), (ci + 1) * (CH // SCENE))
                pool_e = [pdec, pc1, pc3, pc2][ci % 4]
                tag_e = ["psdec", "psc1", "psc3", "psc2"][ci % 4]
                ps = pool_e.tile([24, CH], F32, tag=tag_e, name="psR")
                for slot in range(NSLOT):
                    nc.tensor.matmul(
                        ps[:], w["relA"][:, slot * 24:(slot + 1) * 24],
                        S_all[:, slot, sl],
                        start=(slot == 0), stop=(slot == NSLOT - 1))
                rmx = (sbRelMx[:, ssl].unsqueeze(2)
                       .broadcast_to((24, CH // SCENE, SCENE)))
                cmb = KNOBS['combine']
                eng(cmb[ci % len(cmb)]).tensor_add(relout[:, sl], ps[:], rmx)
                nc.sync.dma_start(out=rels_d[:, sl], in_=relout[:, sl])

    nc.compile()   # bacc passes: split multi-waits into EventSemaphores etc.
    return nc


def _numpy_fallback(obs_traj, W_se, b_se, v1, g1, b1, v2, g2, b2, v3, g3, b3,
                    W_hp, b_hp, seq_start_end, seq_len):
    """Exact numpy implementation for inputs the device kernel wasn't built
    for (non-uniform segments / different seq_len)."""
    batch = obs_traj.shape[1]
    nseg = seq_start_end.shape[0]
    seg = np.searchsorted(seq_start_end[:, 0], np.arange(batch),
                          side="right") - 1

    def wn(v, g):
        n = np.sqrt((v * v).sum(axis=(1, 2)))
        return v * (g / n)[:, None, None]

    w1, w2, w3 = wn(v1, g1), wn(v2, g2), wn(v3, g3)

    def conv(x, w, b):
        O = w.shape[0]
        Tn = x.shape[2]
        out = np.zeros((x.shape[0], O, Tn - 2), np.float32)
        for t in range(Tn - 2):
            for k in range(3):
                out[:, :, t] += x[:, :, t + k] @ w[:, :, k].T
        return np.maximum(out + b[None, :, None], 0)

    emb = obs_traj @ W_se.T + b_se
    obs_emb = np.transpose(emb, (1, 2, 0)).copy()
    rels = []
    for _ in range(int(seq_len)):
        c3 = conv(conv(conv(obs_emb, w1, b1), w2, b2), w3, b3)
        s = c3.reshape(batch, 64)
        mx = np.full((nseg, 64), -np.inf, np.float32)
        np.maximum.at(mx, seg, s)
        st = np.concatenate([s, mx[seg]], axis=1)
        rel = st @ W_hp.T + b_hp
        dec = rel @ W_se.T + b_se
        obs_emb = np.concatenate([obs_emb[:, :, 1:], dec[:, :, None]], axis=2)
        rels.append(rel)
    return np.stack(rels).astype(np.float32)


def kernel(obs_traj, last_pos, last_pos_rel, W_se, b_se, v1, g1, b1,
           v2, g2, b2, v3, g3, b3, W_hp, b_hp, seq_start_end, seq_len):
    obs_traj = np.asarray(obs_traj, np.float32)
    seq_start_end = np.asarray(seq_start_end)
    args = [np.asarray(a, np.float32) for a in
            (W_se, b_se, v1, g1, b1, v2, g2, b2, v3, g3, b3, W_hp, b_hp)]

    starts = np.arange(BATCH // SCENE, dtype=np.int64) * SCENE
    uniform = (obs_traj.shape == (T, BATCH, 2)
               and int(seq_len) == SEQ
               and seq_start_end.shape == (BATCH // SCENE, 2)
               and np.array_equal(seq_start_end[:, 0], starts)
               and np.array_equal(seq_start_end[:, 1], starts + SCENE))
    if not uniform:
        return _numpy_fallback(obs_traj, *args, seq_start_end, seq_len)

    if "nc" not in _cache:
        _cache["nc"] = _build_module()
    nc = _cache["nc"]

    wdev = _host_weights(*args)
    obs_t = np.concatenate([obs_traj.transpose(0, 2, 1),
                            np.ones((T, 1, BATCH), np.float32)],
                           axis=1)  # (8, 3, 32768) with ones plane

    in_maps = []
    for core in range(NCORES):
        m = dict(wdev)
        m["obs"] = np.ascontiguousarray(obs_t[:, :, core * B:(core + 1) * B])
        in_maps.append(m)

    res = run_bass_kernel_spmd(nc, in_maps, core_ids=list(range(NCORES)))

    out = np.empty((SEQ, BATCH, 2), np.float32)
    for core in range(NCORES):
        arr = res.results[core]["rels"]          # (24, B)
        for c in range(2):
            out[:, core * B:(core + 1) * B, c] = arr[c::2]
    return out
